# revision 12
# baseline (speedup 1.0000x reference)
"""Trainium2 Bass kernel for a 2-layer GCN encoder (PyG GCNConv semantics).

Strategy (8 NeuronCores, SPMD):
  * Nodes sharded across 8 cores (6250/core); edges partitioned by
    destination shard; weights replicated.
  * Dense layer projections on the local shard; h-tables exchanged with
    chunked AllGathers (S=4 contiguous Shared tables per layer, section
    boundaries graduated small-first/small-last so the first table ships
    early and the last ships with minimal tail).
  * Layer-1 tables are fp8 (e4m3): halves the dominant gather + AllGather
    byte traffic.  Layer-2 tables stay fp16 (fp8 rows would be 128B,
    below dma_gather's 256B row floor).  Aggregation stays in fp32 PSUM.
  * Per-shard aggregation via batched gather DMAs (dma_gather) + one-hot
    selection-matrix matmuls on the TensorEngine.  Gather descriptor
    generation round-robins over all 4 SWDGE queues.
  * Emission is SECTION-major everywhere: all gathers/matmuls for table
    section s (all groups) are emitted before section s+1, so the single
    in-order Pool/engine streams never head-of-line block on a
    not-yet-landed AllGather.  Per-tile partial sums close per section in
    PSUM and accumulate into persistent fp16 SBUF accumulators.
  * Self-loops are NOT gathered: their contribution dinv[d]*h[d] is added
    with one identity matmul per tile over the local h' rows (section-0
    PSUM group, together with the rank-1 bias matmul sqrt(deg)[n] x b[f]).
  * The symmetric norm dinv[src]*dinv[dst] is factored: dinv[src] is
    pre-scaled into the gathered tables (h' = D^-1/2 h), dinv[dst] is a
    per-partition post-scale on the accumulated sum.

All preprocessing (degrees, edge sorting/packing, int16 gather index
tables) happens on the host in numpy inside kernel().
"""

import contextlib
import os
import sys

import numpy as np

for _p in ("/opt/trn_rl_repo", "/root/.axon_site/_ro/trn_rl_repo"):
    if os.path.isdir(_p) and _p not in sys.path:
        sys.path.insert(0, _p)

import concourse.bacc as bacc
import concourse.mybir as mybir
import concourse.tile as tile
from concourse.bass_utils import run_bass_kernel_spmd
from concourse.library_config import mlp as _mlp_lib

P = 128
CORES = 8
GRP = 4  # destination-node tiles per gather group
GCAP = 1024  # max rows per dma_gather call (single-packet ceiling)
NQ = 4  # SWDGE queues: gather desc-gen runs on DSP pair (2q, 2q+1)
SEC_T = [0, 16, 40, 49]  # table-section boundaries in source tiles
S = len(SEC_T) - 1

F8 = mybir.dt.float8e4
F16 = mybir.dt.float16
F32 = mybir.dt.float32
I16 = mybir.dt.int16


def _cdiv(a, b):
    return -(-a // b)


class Plan:
    """Static (cross-core identical) schedule + per-core host arrays.

    Source rows are split into S contiguous sections at tile boundaries
    SEC_T; section s forms gather table s (AllGather chunk s, rows
    c*sec_rows[s] + (r - sec_r[s])).  All tables fit int16 indices.
    Chunk schedule: per group of GRP destination tiles and per section,
    the member tiles' edge runs are packed back-to-back; chunk j gets one
    selection plane per destination tile any core's run overlaps.  Rows
    are padded (gathering table row 0 harmlessly: the one-hot has
    drel=-1 there) only at the section tail, up to the max core's count.
    Self-loops are excluded (identity matmul adds them on-device).
    """

    def __init__(self, n_nodes, edge_src, edge_dst, in_c, hid, out_c):
        assert n_nodes % CORES == 0
        self.n = n_nodes
        self.inc, self.hid, self.outc = in_c, hid, out_c
        self.shard = n_nodes // CORES
        self.tiles = _cdiv(self.shard, P)
        self.shard_pad = self.tiles * P
        self.npad = self.shard_pad * CORES
        assert SEC_T[-1] == self.tiles
        self.sec_r = [t * P for t in SEC_T]  # row boundaries per core
        self.sec_rows = [
            self.sec_r[s + 1] - self.sec_r[s] for s in range(S)
        ]
        for rows in self.sec_rows:
            assert CORES * rows < 32768

        deg = np.bincount(edge_dst, minlength=n_nodes).astype(np.float64) + 1.0
        self.dinv_full = 1.0 / np.sqrt(deg)
        dinv = self.dinv_full

        shard = self.shard
        tiles = self.tiles
        core_of = edge_dst // shard
        sec_r_arr = np.asarray(self.sec_r[1:])  # upper bounds
        percore = []  # (idx16, t, drel, sec)
        cnt = np.zeros((CORES, tiles, S), np.int64)  # [core, tile, sec]
        for c in range(CORES):
            m = core_of == c
            s = edge_src[m]
            d = edge_dst[m]
            sc = s // shard
            r = s % shard
            sec = np.searchsorted(sec_r_arr, r, side="right")
            base = np.asarray([self.sec_r[k] for k in range(S)])
            idx16 = sc * np.asarray(self.sec_rows)[sec] + (r - base[sec])
            dloc = d - c * shard
            t = dloc // P
            drel = dloc % P
            percore.append((idx16, t, drel, sec))
            for k in range(S):
                cnt[c, :, k] += np.bincount(t[sec == k], minlength=tiles)

        # group schedule
        self.groups = []
        chunk_base = 0  # running chunk count (= msg cols / gather rows /128)
        plane_base = 0  # running plane count (= dr_sb cols)
        idxcol_base = 0
        for g0 in range(0, tiles, GRP):
            gt = list(range(g0, min(g0 + GRP, tiles)))
            secs = []
            ch_off = 0
            pl_off = 0
            for sec in range(S):
                ccnt = cnt[:, gt, sec]  # [core, tiles-in-group]
                rows = ccnt.sum(axis=1)  # per-core section rows
                nch = int(_cdiv(int(rows.max()), P)) if rows.max() else 0
                offs = np.zeros((CORES, len(gt) + 1), np.int64)
                offs[:, 1:] = np.cumsum(ccnt, axis=1)
                tiles_of = [[] for _ in range(nch)]
                for j in range(nch):
                    lo, hi = j * P, (j + 1) * P
                    for k, t in enumerate(gt):
                        if ((offs[:, k] < hi) & (offs[:, k + 1] > lo)).any():
                            tiles_of[j].append(k)
                plane_list = []  # (k local-tile, j chunk)
                tile_planes = {k: [] for k in range(len(gt))}
                for k in range(len(gt)):
                    for j in range(nch):
                        if k in tiles_of[j]:
                            tile_planes[k].append((len(plane_list), j))
                            plane_list.append((k, j))
                secs.append(
                    dict(
                        nch=nch,
                        offs=offs,
                        plane_list=plane_list,
                        tile_planes=tile_planes,
                        chunk_off=ch_off,
                        plane_off=pl_off,
                    )
                )
                ch_off += nch
                pl_off += len(plane_list)
            self.groups.append(
                dict(
                    tiles=gt,
                    secs=secs,
                    nch=ch_off,
                    nplanes=pl_off,
                    chunk_base=chunk_base,
                    plane_base=plane_base,
                    idxcol_base=idxcol_base,
                )
            )
            chunk_base += ch_off
            plane_base += pl_off
            idxcol_base += ch_off * P // 16
        self.tot_chunks = chunk_base
        self.tot_planes = plane_base
        self.tot_idxcols = idxcol_base

        # layer-2 AllGather ranges in group units (section tile boundaries
        # are GRP-aligned except the last)
        self.l2r = []
        for s in range(S):
            self.l2r.append((SEC_T[s] // GRP, _cdiv(SEC_T[s + 1], GRP)))

        # per-core flat arrays in schedule order
        self.core_idx = []
        self.core_drel = []
        self.core_dinvc = []
        self.core_sqd = []
        for c in range(CORES):
            idx16, t_arr, drel, secarr = percore[c]
            idx_flat = np.zeros(self.tot_chunks * P, np.int32)
            drel_planes = np.full((self.tot_planes, P), -1.0, np.float32)
            for g in self.groups:
                gt = g["tiles"]
                for sec in range(S):
                    Sd = g["secs"][sec]
                    if not Sd["nch"]:
                        continue
                    msec = secarr == sec
                    base_row = (g["chunk_base"] + Sd["chunk_off"]) * P
                    offs = Sd["offs"][c]
                    for k, t in enumerate(gt):
                        sel = (t_arr == t) & msec
                        kcnt = int(sel.sum())
                        assert kcnt == offs[k + 1] - offs[k]
                        idx_flat[
                            base_row + offs[k] : base_row + offs[k] + kcnt
                        ] = idx16[sel]
                        for pl, j in Sd["tile_planes"][k]:
                            lo = max(int(offs[k]), j * P)
                            hi = min(int(offs[k + 1]), (j + 1) * P)
                            if hi <= lo:
                                continue
                            rows = np.arange(lo, hi)
                            drel_planes[
                                g["plane_base"] + Sd["plane_off"] + pl,
                                rows - j * P,
                            ] = drel[sel][lo - int(offs[k]) : hi - int(offs[k])]
            # wrap idx into the dma_gather SBUF layout [128, cols]: per
            # call, idx i lives at [p, i // 16] for p % 16 == i % 16,
            # replicated 8x over partitions.  Calls are <=GCAP rows.
            blocks = []
            for g in self.groups:
                a = g["chunk_base"] * P
                for sec in range(S):
                    n = g["secs"][sec]["nch"] * P
                    for off in range(0, n, GCAP):
                        nn = min(GCAP, n - off)
                        v = idx_flat[a + off : a + off + nn].reshape(-1, 16).T
                        blocks.append(np.tile(v, (8, 1)))
                    a += n
            idx_sb = (
                np.concatenate(blocks, axis=1).astype(np.int16)
                if blocks
                else np.zeros((P, 0), np.int16)
            )
            assert idx_sb.shape == (P, self.tot_idxcols), idx_sb.shape
            self.core_idx.append(np.ascontiguousarray(idx_sb))
            self.core_drel.append(np.ascontiguousarray(drel_planes.T))
            # per-node scale planes for this shard (pad nodes: dinv=1, sqd=0)
            dshard = np.ones(self.shard_pad, np.float64)
            dshard[:shard] = dinv[c * shard : (c + 1) * shard]
            dc = dshard.reshape(self.tiles, P).T.astype(np.float32)
            self.core_dinvc.append(np.ascontiguousarray(dc))
            sq = np.zeros(self.shard_pad, np.float64)
            sq[:shard] = 1.0 / dinv[c * shard : (c + 1) * shard]
            self.core_sqd.append(sq.astype(np.float16).reshape(1, self.shard_pad))


def _build_nc(plan):
    inc, hid, outc = plan.inc, plan.hid, plan.outc
    ncc = inc // P  # contraction chunks for layer-1 projection
    hcc = hid // P  # contraction chunks for layer-2 projection
    tiles = plan.tiles

    nc = bacc.Bacc("TRN2", num_devices=CORES, num_swdge_queues=NQ)

    xt_d = nc.dram_tensor("xt", [tiles, P, ncc, P], F16, kind="ExternalInput")
    w1_d = nc.dram_tensor("w1t", [P, ncc, hid], F16, kind="ExternalInput")
    w2_d = nc.dram_tensor("w2t", [P, hcc, outc], F16, kind="ExternalInput")
    b1_d = nc.dram_tensor("b1r", [1, hid], F16, kind="ExternalInput")
    b2_d = nc.dram_tensor("b2r", [1, outc], F16, kind="ExternalInput")
    io_d = nc.dram_tensor("iota", [P, P], F32, kind="ExternalInput")
    id_d = nc.dram_tensor("ident", [P, P], F16, kind="ExternalInput")
    ix_d = nc.dram_tensor("idx", [P, plan.tot_idxcols], I16, kind="ExternalInput")
    dr_d = nc.dram_tensor("dstrel", [P, plan.tot_planes], F32, kind="ExternalInput")
    dv_d = nc.dram_tensor("dinvc", [P, tiles], F32, kind="ExternalInput")
    sq_d = nc.dram_tensor("sqd", [1, plan.shard_pad], F16, kind="ExternalInput")
    out_d = nc.dram_tensor("out", [plan.shard_pad, outc], F32, kind="ExternalOutput")

    h1_sh = nc.dram_tensor("h1_shard", [plan.shard_pad, hid], F8)
    h2_sh = nc.dram_tensor("h2_shard", [plan.shard_pad, outc], F16)
    h1_f = [
        nc.dram_tensor(
            f"h1_f{s}", [CORES * plan.sec_rows[s], hid], F8, addr_space="Shared"
        )
        for s in range(S)
    ]
    h2_f = [
        nc.dram_tensor(
            f"h2_f{s}", [CORES * plan.sec_rows[s], outc], F16, addr_space="Shared"
        )
        for s in range(S)
    ]

    st = contextlib.ExitStack()
    idx_sb = st.enter_context(nc.sbuf_tensor("idx_sb", [P, plan.tot_idxcols], I16))
    dr_sb = st.enter_context(nc.sbuf_tensor("dr_sb", [P, plan.tot_planes], F32))
    io_sb = st.enter_context(nc.sbuf_tensor("io_sb", [P, P], F32))
    id_sb = st.enter_context(nc.sbuf_tensor("id_sb", [P, P], F16))
    w1_sb = st.enter_context(nc.sbuf_tensor("w1_sb", [P, ncc, hid], F16))
    w2_sb = st.enter_context(nc.sbuf_tensor("w2_sb", [P, hcc, outc], F16))
    b1_sb = st.enter_context(nc.sbuf_tensor("b1_sb", [1, hid], F16))
    b2_sb = st.enter_context(nc.sbuf_tensor("b2_sb", [1, outc], F16))
    dv_sb = st.enter_context(nc.sbuf_tensor("dv_sb", [P, tiles], F32))
    sq_sb = st.enter_context(nc.sbuf_tensor("sq_sb", [1, plan.shard_pad], F16))
    # persistent per-tile accumulators (fp16): sections close in PSUM and
    # sum here, so no PSUM tile ever waits for a late AllGather.
    acc1 = st.enter_context(nc.sbuf_tensor("acc1", [P, tiles, hid], F16))
    acc2 = st.enter_context(nc.sbuf_tensor("acc2", [P, tiles, outc], F16))

    nc.gpsimd.load_library(_mlp_lib)

    # one gpsimd register per distinct gather count
    _regs = {}

    def _nreg(v):
        if v not in _regs:
            _regs[v] = nc.gpsimd.to_reg(v)
        return _regs[v]

    for g in plan.groups:
        for sec in range(S):
            n = g["secs"][sec]["nch"] * P
            for off in range(0, n, GCAP):
                _nreg(min(GCAP, n - off))

    # idx column offset per (group, sec)
    icol = {}
    for gi, g in enumerate(plan.groups):
        ic = g["idxcol_base"]
        for sec in range(S):
            icol[(gi, sec)] = ic
            ic += g["secs"][sec]["nch"] * P // 16

    _qctr = [0]

    def _emit_gather(msg, nch, table, ic0, elem):
        """Gather nch*P rows into msg[:, 0:nch, :] in <=GCAP pieces.

        Calls round-robin over the SWDGE queues so descriptor generation
        runs on all four Q7 DSP pairs concurrently."""
        n = nch * P
        ic = ic0
        for off in range(0, n, GCAP):
            nn = min(GCAP, n - off)
            nc.gpsimd.dma_gather(
                msg[:, off // P : (off + nn) // P, :],
                table,
                idx_sb[:, ic : ic + nn // 16],
                nn,
                _nreg(nn),
                elem,
                queue_num=_qctr[0] % NQ,
            )
            _qctr[0] += 1
            ic += nn // 16

    # ctx0: resident loads (own context so its exit barrier orders them
    # before every later consumer on every engine).
    with tile.TileContext(nc):
        nc.sync.dma_start(idx_sb[:, :], ix_d[:, :])
        nc.sync.dma_start(dr_sb[:, :], dr_d[:, :])
        nc.sync.dma_start(io_sb[:, :], io_d[:, :])
        nc.sync.dma_start(id_sb[:, :], id_d[:, :])
        nc.sync.dma_start(w1_sb[:, :, :], w1_d[:, :, :])
        nc.sync.dma_start(w2_sb[:, :, :], w2_d[:, :, :])
        nc.sync.dma_start(b1_sb[:, :], b1_d[:, :])
        nc.sync.dma_start(b2_sb[:, :], b2_d[:, :])
        nc.sync.dma_start(dv_sb[:, :], dv_d[:, :])
        nc.sync.dma_start(sq_sb[:, :], sq_d[:, :])

    # Fused context: everything in one TileContext so the scheduler
    # overlaps collectives and gathers with compute across phase
    # boundaries.
    with tile.TileContext(nc) as tc:
      with (
        tc.tile_pool(name="pA", bufs=2) as pA,
        tc.tile_pool(name="msgB", bufs=5) as msgB,
        tc.tile_pool(name="locB", bufs=2) as locB,
        tc.tile_pool(name="sB", bufs=3) as sB,
        tc.tile_pool(name="zB", bufs=3) as zB,
        tc.tile_pool(name="h2B", bufs=3) as h2B,
        tc.tile_pool(name="psB", bufs=3, space="PSUM") as psB,
        tc.tile_pool(name="psT", bufs=2, space="PSUM") as psT,
        tc.tile_pool(name="psH", bufs=3, space="PSUM") as psH,
        tc.tile_pool(name="msgC", bufs=5) as msgC,
        tc.tile_pool(name="locC", bufs=2) as locC,
        tc.tile_pool(name="sC", bufs=3) as sC,
        tc.tile_pool(name="oC", bufs=3) as oC,
      ):
        psA = psB   # same [P, hid] f32 tiles; stay within 8 PSUM banks
        psC = psH   # same [P, outc] f32 tiles

        # ---- phase A: layer-1 dense projection h1' = D^-1/2 (x @ W1),
        # fp8 tables; each section AllGathers as soon as it is written.
        for s in range(S):
            for t0 in range(SEC_T[s], SEC_T[s + 1], 4):
                nt = min(4, SEC_T[s + 1] - t0)
                xa = pA.tile([P, nt, ncc, P], F16, tag="xa")
                nc.sync.dma_start(
                    xa[:, :, :, :],
                    xt_d[t0 : t0 + nt, :, :, :].rearrange("t p c n -> p t c n"),
                )
                h1t = pA.tile([P, nt, hid], F8, tag="h1t")
                for k in range(nt):
                    ps = psA.tile([P, hid], F32, tag="psAgg")
                    for cc in range(ncc):
                        nc.tensor.matmul(
                            ps[:, :],
                            lhsT=xa[:, k, cc, :],
                            rhs=w1_sb[:, cc, :],
                            start=(cc == 0),
                            stop=(cc == ncc - 1),
                        )
                    nc.scalar.activation(
                        h1t[:, k, :],
                        ps[:, :],
                        mybir.ActivationFunctionType.Copy,
                        scale=dv_sb[:, t0 + k : t0 + k + 1],
                    )
                nc.sync.dma_start(
                    h1_sh[t0 * P : (t0 + nt) * P, :].rearrange(
                        "(t p) f -> p t f", p=P
                    ),
                    h1t[:, :, :],
                )
            # section AllGather fires as soon as its rows are projected.
            # Collectives move the same bytes as f32 views (wider element
            # = fewer elements on the element-rate-bound CC path).
            nc.gpsimd.collective_compute(
                "AllGather",
                mybir.AluOpType.bypass,
                replica_groups=[list(range(CORES))],
                ins=[h1_sh[plan.sec_r[s] : plan.sec_r[s + 1], :].bitcast(F32)],
                outs=[h1_f[s][:, :].bitcast(F32)],
            )

        # ---- phase B/C/D: layer-1 gathers + aggregation, GROUP-major so
        # early groups complete early (their layer-2 AllGather ships while
        # later groups still gather); PSUM closes per section into acc1 so
        # nothing holds PSUM across a not-yet-landed section.  Relu +
        # layer-2 projection per group; layer-2 AllGather per range.
        msgs1 = {}

        def l1_sec_group(s, gi, g, loc):
            Sd = g["secs"][s]
            for k, t in enumerate(g["tiles"]):
                tp = Sd["tile_planes"][k]
                if not tp and s > 0:
                    continue
                ps = psB.tile([P, hid], F32, tag="psAgg")
                npl = len(tp)
                if npl:
                    c0 = g["plane_base"] + Sd["plane_off"] + tp[0][0]
                    assert tp[-1][0] - tp[0][0] == npl - 1  # contiguous
                    Stile = sB.tile([P, npl, P], F16, tag="S1")
                    nc.vector.tensor_tensor(
                        out=Stile[:, :, :],
                        in0=io_sb[:, None, :].to_broadcast([P, npl, P]),
                        in1=dr_sb[:, c0 : c0 + npl][:, :, None].to_broadcast(
                            [P, npl, P]
                        ),
                        op=mybir.AluOpType.is_equal,
                    )
                    msg = msgs1[(gi, s)]
                    for i, (_pl, ch) in enumerate(tp):
                        nc.tensor.matmul(
                            ps[:, :],
                            lhsT=Stile[:, i, :],
                            rhs=msg[:, ch, :],
                            start=(i == 0),
                            stop=(s > 0 and i == npl - 1),
                        )
                if s == 0:
                    # self-loop: += dinv[d]*h[d] (local h' rows)
                    nc.tensor.matmul(
                        ps[:, :],
                        lhsT=id_sb[:, :],
                        rhs=loc[:, k, :],
                        start=(npl == 0),
                        stop=False,
                    )
                    # rank-1 bias: += sqrt(deg)[n] * b[f]
                    nc.tensor.matmul(
                        ps[:, :],
                        lhsT=sq_sb[0:1, t * P : (t + 1) * P],
                        rhs=b1_sb[0:1, :],
                        start=False,
                        stop=True,
                    )
                    nc.scalar.activation(
                        acc1[:, t, :],
                        ps[:, :],
                        mybir.ActivationFunctionType.Copy,
                    )
                else:
                    nc.vector.tensor_tensor(
                        out=acc1[:, t, :],
                        in0=acc1[:, t, :],
                        in1=ps[:, :],
                        op=mybir.AluOpType.add,
                    )

        def l2_proj_group(g):
            for k, t in enumerate(g["tiles"]):
                # z1 = relu(dinv[n] * acc1)  [node, f] fp16
                z1 = zB.tile([P, hid], F16, tag="z1")
                nc.vector.tensor_scalar(
                    out=z1[:, :],
                    in0=acc1[:, t, :],
                    scalar1=dv_sb[:, t : t + 1],
                    scalar2=0.0,
                    op0=mybir.AluOpType.mult,
                    op1=mybir.AluOpType.max,
                )
                # transpose z1 -> z1T for the layer-2 contraction
                zt_ps = psT.tile([P, hcc, P], F16, tag="ztps")
                for h in range(hcc):
                    nc.tensor.transpose(
                        zt_ps[:, h, :],
                        z1[:, h * P : (h + 1) * P],
                        id_sb[:, :],
                    )
                zt = zB.tile([P, hcc, P], F16, tag="zt")
                nc.scalar.activation(
                    zt[:, :, :],
                    zt_ps[:, :, :],
                    mybir.ActivationFunctionType.Copy,
                )
                hps = psH.tile([P, outc], F32, tag="hps")
                for cc in range(hcc):
                    nc.tensor.matmul(
                        hps[:, :],
                        lhsT=zt[:, cc, :],
                        rhs=w2_sb[:, cc, :],
                        start=(cc == 0),
                        stop=(cc == hcc - 1),
                    )
                # h2' = dinv[n] * (z1 @ W2)
                h2t = h2B.tile([P, outc], F16, tag="h2t")
                nc.scalar.activation(
                    h2t[:, :],
                    hps[:, :],
                    mybir.ActivationFunctionType.Copy,
                    scale=dv_sb[:, t : t + 1],
                )
                nc.sync.dma_start(h2_sh[t * P : (t + 1) * P, :], h2t[:, :])

        for r, (g0, g1) in enumerate(plan.l2r):
            for gi in range(g0, g1):
                g = plan.groups[gi]
                nt = len(g["tiles"])
                t0g = g["tiles"][0]
                for s in range(S):
                    nch = g["secs"][s]["nch"]
                    if not nch:
                        continue
                    msg = msgB.tile([P, nch, hid], F8, tag="msg1")
                    _emit_gather(msg, nch, h1_f[s][:, :], icol[(gi, s)], hid)
                    msgs1[(gi, s)] = msg
                loc = locB.tile([P, nt, hid], F8, tag="loc")
                nc.sync.dma_start(
                    loc[:, :, :],
                    h1_sh[t0g * P : (t0g + nt) * P, :].rearrange(
                        "(t p) f -> p t f", p=P
                    ),
                )
                for s in range(S):
                    l1_sec_group(s, gi, g, loc)
                l2_proj_group(g)
            # layer-2 section AllGather: range r's h2' rows are now
            # projected; the trigger waits only on those DMA writes.
            nc.gpsimd.collective_compute(
                "AllGather",
                mybir.AluOpType.bypass,
                replica_groups=[list(range(CORES))],
                ins=[h2_sh[plan.sec_r[r] : plan.sec_r[r + 1], :].bitcast(F32)],
                outs=[h2_f[r][:, :].bitcast(F32)],
            )

        # ---- phase E/F: layer-2 gathers + aggregation, section-major
        # (all l2 AllGather triggers were already dispatched above);
        # final dinv scale + output store per group after the last
        # section.
        locs2 = {}
        for s in range(S):
            for gi, g in enumerate(plan.groups):
                nch = g["secs"][s]["nch"]
                if not nch:
                    continue
                msg = msgC.tile([P, nch, outc], F16, tag="msg2")
                _emit_gather(msg, nch, h2_f[s][:, :], icol[(gi, s)], outc)
                msgs1[(gi, s, "l2")] = msg
            for gi, g in enumerate(plan.groups):
                nt = len(g["tiles"])
                t0g = g["tiles"][0]
                if s == 0:
                    loc = locC.tile([P, nt, outc], F16, tag="loc2")
                    nc.sync.dma_start(
                        loc[:, :, :],
                        h2_sh[t0g * P : (t0g + nt) * P, :].rearrange(
                            "(t p) f -> p t f", p=P
                        ),
                    )
                    locs2[gi] = loc
                Sd = g["secs"][s]
                for k, t in enumerate(g["tiles"]):
                    tp = Sd["tile_planes"][k]
                    if not tp and s > 0:
                        continue
                    ps = psC.tile([P, outc], F32, tag="hps")
                    npl = len(tp)
                    if npl:
                        c0 = g["plane_base"] + Sd["plane_off"] + tp[0][0]
                        assert tp[-1][0] - tp[0][0] == npl - 1
                        Stile = sC.tile([P, npl, P], F16, tag="S2")
                        nc.vector.tensor_tensor(
                            out=Stile[:, :, :],
                            in0=io_sb[:, None, :].to_broadcast([P, npl, P]),
                            in1=dr_sb[:, c0 : c0 + npl][:, :, None].to_broadcast(
                                [P, npl, P]
                            ),
                            op=mybir.AluOpType.is_equal,
                        )
                        msg = msgs1[(gi, s, "l2")]
                        for i, (_pl, ch) in enumerate(tp):
                            nc.tensor.matmul(
                                ps[:, :],
                                lhsT=Stile[:, i, :],
                                rhs=msg[:, ch, :],
                                start=(i == 0),
                                stop=(s > 0 and i == npl - 1),
                            )
                    if s == 0:
                        nc.tensor.matmul(
                            ps[:, :],
                            lhsT=id_sb[:, :],
                            rhs=locs2[gi][:, k, :],
                            start=(npl == 0),
                            stop=False,
                        )
                        nc.tensor.matmul(
                            ps[:, :],
                            lhsT=sq_sb[0:1, t * P : (t + 1) * P],
                            rhs=b2_sb[0:1, :],
                            start=False,
                            stop=True,
                        )
                        nc.scalar.activation(
                            acc2[:, t, :],
                            ps[:, :],
                            mybir.ActivationFunctionType.Copy,
                        )
                    else:
                        nc.vector.tensor_tensor(
                            out=acc2[:, t, :],
                            in0=acc2[:, t, :],
                            in1=ps[:, :],
                            op=mybir.AluOpType.add,
                        )
                if s == S - 1:
                    for k, t in enumerate(g["tiles"]):
                        ob = oC.tile([P, outc], F32, tag="ob")
                        nc.scalar.activation(
                            ob[:, :],
                            acc2[:, t, :],
                            mybir.ActivationFunctionType.Copy,
                            scale=dv_sb[:, t : t + 1],
                        )
                        nc.sync.dma_start(
                            out_d[t * P : (t + 1) * P, :], ob[:, :]
                        )

    st.close()
    nc.compile()
    return nc


def _make_in_maps(plan, x, W1, b1, W2, b2):
    inc, hid, outc = plan.inc, plan.hid, plan.outc
    ncc, hcc = inc // P, hid // P
    w1t = np.ascontiguousarray(
        W1.reshape(ncc, P, hid).transpose(1, 0, 2).astype(np.float16)
    )
    w2t = np.ascontiguousarray(
        W2.reshape(hcc, P, outc).transpose(1, 0, 2).astype(np.float16)
    )
    b1r = np.ascontiguousarray(b1.astype(np.float16).reshape(1, hid))
    b2r = np.ascontiguousarray(b2.astype(np.float16).reshape(1, outc))
    iota = np.ascontiguousarray(
        np.tile(np.arange(P, dtype=np.float32), (P, 1))
    )
    ident = np.ascontiguousarray(np.eye(P, dtype=np.float16))
    in_maps = []
    for c in range(CORES):
        xs = x[c * plan.shard : (c + 1) * plan.shard].astype(np.float32)
        xs = np.pad(xs, ((0, plan.shard_pad - plan.shard), (0, 0)))
        xt = xs.reshape(plan.tiles, P, ncc, P).transpose(0, 3, 2, 1)
        in_maps.append(
            {
                "xt": np.ascontiguousarray(xt.astype(np.float16)),
                "w1t": w1t,
                "w2t": w2t,
                "b1r": b1r,
                "b2r": b2r,
                "iota": iota,
                "ident": ident,
                "idx": plan.core_idx[c],
                "dstrel": plan.core_drel[c],
                "dinvc": plan.core_dinvc[c],
                "sqd": plan.core_sqd[c],
            }
        )
    return in_maps


_CACHE = {}


def _get_built(x, edge_index, W1, b1, W2, b2):
    n_nodes, in_c = x.shape
    hid = W1.shape[1]
    out_c = W2.shape[1]
    key = (n_nodes, in_c, hid, out_c, hash(edge_index.tobytes()))
    if key not in _CACHE:
        src = np.asarray(edge_index[0], np.int64)
        dst = np.asarray(edge_index[1], np.int64)
        plan = Plan(n_nodes, src, dst, in_c, hid, out_c)
        nc = _build_nc(plan)
        _CACHE[key] = (plan, nc)
    return _CACHE[key]


def run(x, edge_index, W1, b1, W2, b2, trace=False, **spmd_kwargs):
    plan, nc = _get_built(x, edge_index, W1, b1, W2, b2)
    in_maps = _make_in_maps(plan, x, W1, b1, W2, b2)
    res = run_bass_kernel_spmd(
        nc, in_maps, core_ids=list(range(CORES)), trace=trace, **spmd_kwargs
    )
    out = np.concatenate(
        [res.results[c]["out"][: plan.shard] for c in range(CORES)], axis=0
    ).astype(np.float32)
    return out, res


def kernel(**inputs):
    x = np.asarray(inputs["x"], np.float32)
    edge_index = np.asarray(inputs["edge_index"])
    W1 = np.asarray(inputs["W1"], np.float32)
    b1 = np.asarray(inputs["b1"], np.float32)
    W2 = np.asarray(inputs["W2"], np.float32)
    b2 = np.asarray(inputs["b2"], np.float32)
    out, _ = run(x, edge_index, W1, b1, W2, b2)
    return out


# revision 13
# speedup vs baseline: 1.1211x; 1.1211x over previous
"""Trainium2 Bass kernel for a 2-layer GCN encoder (PyG GCNConv semantics).

Strategy (8 NeuronCores, SPMD):
  * Nodes sharded across 8 cores (6250/core); edges partitioned by
    destination shard; weights replicated.
  * Dense layer projections on the local shard; h-tables exchanged with
    chunked AllGathers (two contiguous Shared tables A/B per layer, split
    at local row 3072) that overlap the producing compute; per-shard
    aggregation via batched gather DMAs (dma_gather) + one-hot
    selection-matrix matmuls on the TensorEngine.
  * Gather ordering against the AllGather chunks is handled by Tile's
    dependency tracking (collectives are issued inside the TileContext).
  * Gather descriptor generation is spread over all 4 SWDGE queues so all
    four Q7 DSP pairs generate descriptors concurrently.
  * Edge chunks are packed per (group, A/B section) with tiles sharing
    boundary chunks: a chunk overlapping two destination tiles gets one
    selection plane per (chunk, tile) pair (padding ~3% instead of ~17%).
  * Self-loops are NOT gathered: their contribution dinv[d]*h[d] is added
    with one identity matmul per tile over the local h' rows.
  * The symmetric norm dinv[src]*dinv[dst] is factored: dinv[src] is
    pre-scaled into the gathered tables (h' = D^-1/2 h), dinv[dst] is a
    per-partition post-scale on the aggregated PSUM.
  * Biases are folded in as rank-1 matmuls (sqrt(deg)[n] x b[f]) inside
    the accumulation group, so out = dinv * (R + sqd*b) = dinv*R + b.
  * Tables in fp16 (accumulation in fp32 PSUM).

All preprocessing (degrees, edge sorting/packing, int16 gather index
tables) happens on the host in numpy inside kernel().
"""

import contextlib
import os
import sys

import numpy as np

for _p in ("/opt/trn_rl_repo", "/root/.axon_site/_ro/trn_rl_repo"):
    if os.path.isdir(_p) and _p not in sys.path:
        sys.path.insert(0, _p)

import concourse.bacc as bacc
import concourse.mybir as mybir
import concourse.tile as tile
from concourse.bass_utils import run_bass_kernel_spmd
from concourse.library_config import mlp as _mlp_lib

P = 128
CORES = 8
GRP = 4  # destination-node tiles per gather group
GCAP = 1024  # max rows per dma_gather call (single-packet ceiling)
NQ = 4  # SWDGE queues: gather desc-gen runs on DSP pair (2q, 2q+1)

F8 = mybir.dt.float8e4
F16 = mybir.dt.float16
F32 = mybir.dt.float32
I16 = mybir.dt.int16


def _cdiv(a, b):
    return -(-a // b)


class Plan:
    """Static (cross-core identical) schedule + per-core host arrays.

    Sections: A = sources with local row < split_r (gather table A =
    AllGather chunk A, rows c*split_r + r), B = the rest (table B, rows
    c*(shard_pad-split_r) + r - split_r).  Both tables fit int16.
    Chunk schedule: per group of GRP destination tiles and per section,
    the member tiles' edge runs are packed back-to-back; chunk j gets one
    selection plane per destination tile any core's run overlaps.  Rows
    are padded (gathering table row 0 harmlessly: the one-hot has
    drel=-1 there) only at the section tail, up to the max core's count.
    Self-loops are excluded (identity matmul adds them on-device).
    """

    def __init__(self, n_nodes, edge_src, edge_dst, in_c, hid, out_c):
        assert n_nodes % CORES == 0
        self.n = n_nodes
        self.inc, self.hid, self.outc = in_c, hid, out_c
        self.shard = n_nodes // CORES
        self.tiles = _cdiv(self.shard, P)
        self.shard_pad = self.tiles * P
        self.npad = self.shard_pad * CORES
        self.split_t = 20  # tiles in chunk/table A (GRP-aligned; ships early)
        self.split_r = self.split_t * P
        self.b_rows = self.shard_pad - self.split_r  # per-core rows, table B
        assert CORES * self.split_r < 32768
        assert CORES * self.b_rows < 32768

        deg = np.bincount(edge_dst, minlength=n_nodes).astype(np.float64) + 1.0
        self.dinv_full = 1.0 / np.sqrt(deg)
        dinv = self.dinv_full

        shard = self.shard
        tiles = self.tiles
        core_of = edge_dst // shard
        percore = []  # (idx16, t, drel, isa)
        cnt = np.zeros((CORES, tiles, 2), np.int64)  # [core, tile, sec]
        for c in range(CORES):
            m = core_of == c
            s = edge_src[m]
            d = edge_dst[m]
            sc = s // shard
            r = s % shard
            isa = r < self.split_r
            idx16 = np.where(
                isa, sc * self.split_r + r, sc * self.b_rows + r - self.split_r
            )
            dloc = d - c * shard
            t = dloc // P
            drel = dloc % P
            percore.append((idx16, t, drel, isa))
            for sec in (0, 1):
                msec = isa if sec == 0 else ~isa
                cnt[c, :, sec] = np.bincount(t[msec], minlength=tiles)

        # group schedule
        self.groups = []
        chunk_base = 0  # running chunk count (= msg cols / gather rows /128)
        plane_base = 0  # running plane count (= dr_sb cols)
        idxcol_base = 0
        for g0 in range(0, tiles, GRP):
            gt = list(range(g0, min(g0 + GRP, tiles)))
            secs = []
            for sec in (0, 1):
                ccnt = cnt[:, gt, sec]  # [core, tiles-in-group]
                rows = ccnt.sum(axis=1)  # per-core section rows
                nch = int(_cdiv(int(rows.max()), P)) if rows.max() else 0
                offs = np.zeros((CORES, len(gt) + 1), np.int64)
                offs[:, 1:] = np.cumsum(ccnt, axis=1)
                tiles_of = [[] for _ in range(nch)]
                for j in range(nch):
                    lo, hi = j * P, (j + 1) * P
                    for k, t in enumerate(gt):
                        if ((offs[:, k] < hi) & (offs[:, k + 1] > lo)).any():
                            tiles_of[j].append(k)
                plane_list = []  # (k local-tile, j chunk)
                tile_planes = {k: [] for k in range(len(gt))}
                for k in range(len(gt)):
                    for j in range(nch):
                        if k in tiles_of[j]:
                            tile_planes[k].append((len(plane_list), j))
                            plane_list.append((k, j))
                secs.append(
                    dict(
                        nch=nch,
                        offs=offs,
                        plane_list=plane_list,
                        tile_planes=tile_planes,
                        chunk_off=0,
                        plane_off=0,
                    )
                )
            secs[1]["chunk_off"] = secs[0]["nch"]
            secs[1]["plane_off"] = len(secs[0]["plane_list"])
            nch_g = secs[0]["nch"] + secs[1]["nch"]
            npl_g = len(secs[0]["plane_list"]) + len(secs[1]["plane_list"])
            self.groups.append(
                dict(
                    tiles=gt,
                    secs=secs,
                    nch=nch_g,
                    nplanes=npl_g,
                    chunk_base=chunk_base,
                    plane_base=plane_base,
                    idxcol_base=idxcol_base,
                )
            )
            chunk_base += nch_g
            plane_base += npl_g
            idxcol_base += nch_g * P // 16
        self.tot_chunks = chunk_base
        self.tot_planes = plane_base
        self.tot_idxcols = idxcol_base

        # per-core flat arrays in schedule order
        self.core_idx = []
        self.core_drel = []
        self.core_dinvc = []
        self.core_sqd = []
        for c in range(CORES):
            idx16, t_arr, drel, isa = percore[c]
            idx_flat = np.zeros(self.tot_chunks * P, np.int32)
            drel_planes = np.full((self.tot_planes, P), -1.0, np.float32)
            for g in self.groups:
                gt = g["tiles"]
                for sec in (0, 1):
                    S = g["secs"][sec]
                    if not S["nch"]:
                        continue
                    msec = isa if sec == 0 else ~isa
                    base_row = (g["chunk_base"] + S["chunk_off"]) * P
                    offs = S["offs"][c]
                    for k, t in enumerate(gt):
                        sel = (t_arr == t) & msec
                        kcnt = int(sel.sum())
                        assert kcnt == offs[k + 1] - offs[k]
                        idx_flat[
                            base_row + offs[k] : base_row + offs[k] + kcnt
                        ] = idx16[sel]
                        for pl, j in S["tile_planes"][k]:
                            lo = max(int(offs[k]), j * P)
                            hi = min(int(offs[k + 1]), (j + 1) * P)
                            if hi <= lo:
                                continue
                            rows = np.arange(lo, hi)
                            drel_planes[
                                g["plane_base"] + S["plane_off"] + pl,
                                rows - j * P,
                            ] = drel[sel][lo - int(offs[k]) : hi - int(offs[k])]
            # wrap idx into the dma_gather SBUF layout [128, cols]: per
            # call, idx i lives at [p, i // 16] for p % 16 == i % 16,
            # replicated 8x over partitions.  Calls are <=GCAP rows.
            blocks = []
            for g in self.groups:
                a = g["chunk_base"] * P
                for sec in (0, 1):
                    n = g["secs"][sec]["nch"] * P
                    for off in range(0, n, GCAP):
                        nn = min(GCAP, n - off)
                        v = idx_flat[a + off : a + off + nn].reshape(-1, 16).T
                        blocks.append(np.tile(v, (8, 1)))
                    a += n
            idx_sb = (
                np.concatenate(blocks, axis=1).astype(np.int16)
                if blocks
                else np.zeros((P, 0), np.int16)
            )
            assert idx_sb.shape == (P, self.tot_idxcols), idx_sb.shape
            self.core_idx.append(np.ascontiguousarray(idx_sb))
            self.core_drel.append(np.ascontiguousarray(drel_planes.T))
            # per-node scale planes for this shard (pad nodes: dinv=1, sqd=0)
            dshard = np.ones(self.shard_pad, np.float64)
            dshard[:shard] = dinv[c * shard : (c + 1) * shard]
            dc = dshard.reshape(self.tiles, P).T.astype(np.float32)
            self.core_dinvc.append(np.ascontiguousarray(dc))
            sq = np.zeros(self.shard_pad, np.float64)
            sq[:shard] = 1.0 / dinv[c * shard : (c + 1) * shard]
            self.core_sqd.append(sq.astype(np.float16).reshape(1, self.shard_pad))


def _build_nc(plan):
    inc, hid, outc = plan.inc, plan.hid, plan.outc
    ncc = inc // P  # contraction chunks for layer-1 projection
    hcc = hid // P  # contraction chunks for layer-2 projection
    tiles = plan.tiles
    spl_r = plan.split_r
    spl_t = plan.split_t

    nc = bacc.Bacc("TRN2", num_devices=CORES, num_swdge_queues=NQ)

    xt_d = nc.dram_tensor("xt", [tiles, P, ncc, P], F16, kind="ExternalInput")
    w1_d = nc.dram_tensor("w1t", [P, ncc, hid], F16, kind="ExternalInput")
    w2_d = nc.dram_tensor("w2t", [P, hcc, outc], F16, kind="ExternalInput")
    b1_d = nc.dram_tensor("b1r", [1, hid], F16, kind="ExternalInput")
    b2_d = nc.dram_tensor("b2r", [1, outc], F16, kind="ExternalInput")
    io_d = nc.dram_tensor("iota", [P, P], F32, kind="ExternalInput")
    id_d = nc.dram_tensor("ident", [P, P], F16, kind="ExternalInput")
    ix_d = nc.dram_tensor("idx", [P, plan.tot_idxcols], I16, kind="ExternalInput")
    dr_d = nc.dram_tensor("dstrel", [P, plan.tot_planes], F32, kind="ExternalInput")
    dv_d = nc.dram_tensor("dinvc", [P, tiles], F32, kind="ExternalInput")
    sq_d = nc.dram_tensor("sqd", [1, plan.shard_pad], F16, kind="ExternalInput")
    out_d = nc.dram_tensor("out", [plan.shard_pad, outc], F32, kind="ExternalOutput")

    h1_sh = nc.dram_tensor("h1_shard", [plan.shard_pad, hid], F8)
    h1_fa = nc.dram_tensor("h1_fa", [CORES * spl_r, hid], F8, addr_space="Shared")
    h1_fb = nc.dram_tensor("h1_fb", [CORES * plan.b_rows, hid], F8, addr_space="Shared")
    h2_sh = nc.dram_tensor("h2_shard", [plan.shard_pad, outc], F16)
    h2_fa = nc.dram_tensor("h2_fa", [CORES * spl_r, outc], F16, addr_space="Shared")
    h2_fb = nc.dram_tensor("h2_fb", [CORES * plan.b_rows, outc], F16, addr_space="Shared")

    st = contextlib.ExitStack()
    idx_sb = st.enter_context(nc.sbuf_tensor("idx_sb", [P, plan.tot_idxcols], I16))
    dr_sb = st.enter_context(nc.sbuf_tensor("dr_sb", [P, plan.tot_planes], F32))
    io_sb = st.enter_context(nc.sbuf_tensor("io_sb", [P, P], F32))
    id_sb = st.enter_context(nc.sbuf_tensor("id_sb", [P, P], F16))
    w1_sb = st.enter_context(nc.sbuf_tensor("w1_sb", [P, ncc, hid], F16))
    w2_sb = st.enter_context(nc.sbuf_tensor("w2_sb", [P, hcc, outc], F16))
    b1_sb = st.enter_context(nc.sbuf_tensor("b1_sb", [1, hid], F16))
    b2_sb = st.enter_context(nc.sbuf_tensor("b2_sb", [1, outc], F16))
    dv_sb = st.enter_context(nc.sbuf_tensor("dv_sb", [P, tiles], F32))
    sq_sb = st.enter_context(nc.sbuf_tensor("sq_sb", [1, plan.shard_pad], F16))

    nc.gpsimd.load_library(_mlp_lib)

    # one gpsimd register per distinct gather count
    _regs = {}

    def _nreg(v):
        if v not in _regs:
            _regs[v] = nc.gpsimd.to_reg(v)
        return _regs[v]

    for g in plan.groups:
        for sec in (0, 1):
            n = g["secs"][sec]["nch"] * P
            for off in range(0, n, GCAP):
                _nreg(min(GCAP, n - off))

    _qctr = [0]

    def _emit_gather(msg, ch0, nch, table, ic0, elem):
        """Gather nch*P rows into msg[:, ch0:ch0+nch, :] in <=GCAP pieces.

        Calls round-robin over the SWDGE queues so descriptor generation
        runs on all four Q7 DSP pairs concurrently."""
        n = nch * P
        ic = ic0
        for off in range(0, n, GCAP):
            nn = min(GCAP, n - off)
            nc.gpsimd.dma_gather(
                msg[:, ch0 + off // P : ch0 + (off + nn) // P, :],
                table,
                idx_sb[:, ic : ic + nn // 16],
                nn,
                _nreg(nn),
                elem,
                queue_num=_qctr[0] % NQ,
            )
            _qctr[0] += 1
            ic += nn // 16
        return ic

    def _emit_group_gathers(g, msg, ta, tb, elem):
        ic = g["idxcol_base"]
        an = g["secs"][0]["nch"]
        bn = g["secs"][1]["nch"]
        if an:
            ic = _emit_gather(msg, 0, an, ta, ic, elem)
        if bn:
            _emit_gather(msg, an, bn, tb, ic, elem)

    def _emit_agg(g, msg, loc, sB, psPool, width, b_sb, ptag):
        """Per-tile one-hot aggregation matmuls for group g (+ self-loop
        identity matmul over the local h' rows + rank-1 bias).

        Returns list of (tile_global, psum_tile)."""
        out = []
        for k, t in enumerate(g["tiles"]):
            planes = []  # (plane_col absolute, chunk col within group msg)
            for sec in (0, 1):
                S = g["secs"][sec]
                for pl, j in S["tile_planes"][k]:
                    planes.append(
                        (
                            g["plane_base"] + S["plane_off"] + pl,
                            S["chunk_off"] + j,
                        )
                    )
            npl = len(planes)
            Stile = sB.tile([P, max(npl, 1), P], F16, tag="S")
            pos = 0
            for sec in (0, 1):
                S = g["secs"][sec]
                tp = S["tile_planes"][k]
                if not tp:
                    continue
                c0 = g["plane_base"] + S["plane_off"] + tp[0][0]
                n = len(tp)
                assert tp[-1][0] - tp[0][0] == n - 1  # contiguous
                nc.vector.tensor_tensor(
                    out=Stile[:, pos : pos + n, :],
                    in0=io_sb[:, None, :].to_broadcast([P, n, P]),
                    in1=dr_sb[:, c0 : c0 + n][:, :, None].to_broadcast(
                        [P, n, P]
                    ),
                    op=mybir.AluOpType.is_equal,
                )
                pos += n
            ps = psPool.tile([P, width], F32, tag=ptag)
            for i, (_pl, ch) in enumerate(planes):
                nc.tensor.matmul(
                    ps[:, :],
                    lhsT=Stile[:, i, :],
                    rhs=msg[:, ch, :],
                    start=(i == 0),
                    stop=False,
                )
            # self-loop: += dinv[d]*h[d] (local h' rows, identity matmul)
            nc.tensor.matmul(
                ps[:, :],
                lhsT=id_sb[:, :],
                rhs=loc[:, k, :],
                start=(npl == 0),
                stop=False,
            )
            # rank-1 bias: += sqrt(deg)[n] * b[f]
            nc.tensor.matmul(
                ps[:, :],
                lhsT=sq_sb[0:1, t * P : (t + 1) * P],
                rhs=b_sb[0:1, :],
                start=False,
                stop=True,
            )
            out.append((t, ps))
        return out

    # ctx0: resident loads (own context so its exit barrier orders them
    # before every later consumer on every engine).
    with tile.TileContext(nc):
        nc.sync.dma_start(idx_sb[:, :], ix_d[:, :])
        nc.sync.dma_start(dr_sb[:, :], dr_d[:, :])
        nc.sync.dma_start(io_sb[:, :], io_d[:, :])
        nc.sync.dma_start(id_sb[:, :], id_d[:, :])
        nc.sync.dma_start(w1_sb[:, :, :], w1_d[:, :, :])
        nc.sync.dma_start(w2_sb[:, :, :], w2_d[:, :, :])
        nc.sync.dma_start(b1_sb[:, :], b1_d[:, :])
        nc.sync.dma_start(b2_sb[:, :], b2_d[:, :])
        nc.sync.dma_start(dv_sb[:, :], dv_d[:, :])
        nc.sync.dma_start(sq_sb[:, :], sq_d[:, :])

    # Fused context: dense projection, chunked AllGathers, both
    # aggregation layers all in one TileContext so the scheduler overlaps
    # collectives and gathers with compute across phase boundaries.
    with tile.TileContext(nc) as tc:
      with (
        tc.tile_pool(name="pA", bufs=2) as pA,
        tc.tile_pool(name="msgB", bufs=2) as msgB,
        tc.tile_pool(name="locB", bufs=2) as locB,
        tc.tile_pool(name="sB", bufs=2) as sB,
        tc.tile_pool(name="zB", bufs=2) as zB,
        tc.tile_pool(name="h2B", bufs=3) as h2B,
        tc.tile_pool(name="psB", bufs=3, space="PSUM") as psB,
        tc.tile_pool(name="psT", bufs=2, space="PSUM") as psT,
        tc.tile_pool(name="psH", bufs=3, space="PSUM") as psH,
        tc.tile_pool(name="msgC", bufs=3) as msgC,
        tc.tile_pool(name="locC", bufs=2) as locC,
        tc.tile_pool(name="sC", bufs=3) as sC,
        tc.tile_pool(name="oC", bufs=3) as oC,
      ):
        psA = psB   # same [P, hid] f32 tiles; stay within 8 PSUM banks
        psC = psH   # same [P, outc] f32 tiles
        # layer-1 dense projection h1' = D^-1/2 (x @ W1), 2 tiles/step;
        # each half's h1 rows AllGather as soon as they are written.
        for hi, (s0, s1) in enumerate([(0, spl_t), (spl_t, tiles)]):
                for t0 in range(s0, s1, 4):
                    nt = min(4, s1 - t0)
                    xa = pA.tile([P, nt, ncc, P], F16, tag="xa")
                    nc.sync.dma_start(xa[:, :, :, :], xt_d[t0 : t0 + nt, :, :, :].rearrange("t p c n -> p t c n"))
                    h1t = pA.tile([P, nt, hid], F8, tag="h1t")
                    for k in range(nt):
                        ps = psA.tile([P, hid], F32, tag="psAgg")
                        for cc in range(ncc):
                            nc.tensor.matmul(
                                ps[:, :],
                                lhsT=xa[:, k, cc, :],
                                rhs=w1_sb[:, cc, :],
                                start=(cc == 0),
                                stop=(cc == ncc - 1),
                            )
                        nc.scalar.activation(
                            h1t[:, k, :],
                            ps[:, :],
                            mybir.ActivationFunctionType.Copy,
                            scale=dv_sb[:, t0 + k : t0 + k + 1],
                        )
                    nc.sync.dma_start(
                        h1_sh[t0 * P : (t0 + nt) * P, :].rearrange(
                            "(t p) f -> p t f", p=P
                        ),
                        h1t[:, :, :],
                    )
                nc.gpsimd.collective_compute(
                    "AllGather",
                    mybir.AluOpType.bypass,
                    replica_groups=[list(range(CORES))],
                    ins=[h1_sh[s0 * P : s1 * P, :].bitcast(F32)],
                    outs=[(h1_fa if hi == 0 else h1_fb)[:, :].bitcast(F32)],
                )

        # layer-1 aggregate + relu, then layer-2 dense projection; the
        # first part's h2 rows AllGather while the second part computes.
        gsplit = spl_t // GRP
        assert plan.groups[gsplit]["tiles"][0] == spl_t
        for hi, (gs0, gs1) in enumerate([(0, gsplit), (gsplit, len(plan.groups))]):
            for g in plan.groups[gs0:gs1]:
                nt = len(g["tiles"])
                t0 = g["tiles"][0]
                msg = msgB.tile([P, g["nch"], hid], F8, tag="msg")
                _emit_group_gathers(g, msg, h1_fa[:, :], h1_fb[:, :], hid)
                loc = locB.tile([P, nt, hid], F8, tag="loc")
                nc.sync.dma_start(
                    loc[:, :, :],
                    h1_sh[t0 * P : (t0 + nt) * P, :].rearrange(
                        "(t p) f -> p t f", p=P
                    ),
                )
                for t, ps in _emit_agg(g, msg, loc, sB, psB, hid, b1_sb, "psAgg"):
                    # z1 = relu(dinv[n] * ps)  [node, f] fp16
                    z1 = zB.tile([P, hid], F16, tag="z1")
                    nc.vector.tensor_scalar(
                        out=z1[:, :],
                        in0=ps[:, :],
                        scalar1=dv_sb[:, t : t + 1],
                        scalar2=0.0,
                        op0=mybir.AluOpType.mult,
                        op1=mybir.AluOpType.max,
                    )
                    # transpose z1 -> z1T for the layer-2 contraction
                    zt_ps = psT.tile([P, hcc, P], F16, tag="ztps")
                    for h in range(hcc):
                        nc.tensor.transpose(
                            zt_ps[:, h, :],
                            z1[:, h * P : (h + 1) * P],
                            id_sb[:, :],
                        )
                    zt = zB.tile([P, hcc, P], F16, tag="zt")
                    nc.scalar.activation(
                        zt[:, :, :],
                        zt_ps[:, :, :],
                        mybir.ActivationFunctionType.Copy,
                    )
                    hps = psH.tile([P, outc], F32, tag="hps")
                    for cc in range(hcc):
                        nc.tensor.matmul(
                            hps[:, :],
                            lhsT=zt[:, cc, :],
                            rhs=w2_sb[:, cc, :],
                            start=(cc == 0),
                            stop=(cc == hcc - 1),
                        )
                    # h2' = dinv[n] * (z1 @ W2)
                    h2t = h2B.tile([P, outc], F16, tag="h2t")
                    nc.scalar.activation(
                        h2t[:, :],
                        hps[:, :],
                        mybir.ActivationFunctionType.Copy,
                        scale=dv_sb[:, t : t + 1],
                    )
                    nc.sync.dma_start(h2_sh[t * P : (t + 1) * P, :], h2t[:, :])
            r0 = plan.groups[gs0]["tiles"][0] * P
            r1 = (plan.groups[gs1 - 1]["tiles"][-1] + 1) * P
            nc.gpsimd.collective_compute(
                "AllGather",
                mybir.AluOpType.bypass,
                replica_groups=[list(range(CORES))],
                ins=[h2_sh[r0:r1, :].bitcast(F32)],
                outs=[(h2_fa if hi == 0 else h2_fb)[:, :].bitcast(F32)],
            )

        # layer-2 aggregate + bias -> output
        if True:
            for g in plan.groups:
                nt = len(g["tiles"])
                t0 = g["tiles"][0]
                msg = msgC.tile([P, g["nch"], outc], F16, tag="msg2")
                _emit_group_gathers(g, msg, h2_fa[:, :], h2_fb[:, :], outc)
                loc = locC.tile([P, nt, outc], F16, tag="loc2")
                nc.sync.dma_start(
                    loc[:, :, :],
                    h2_sh[t0 * P : (t0 + nt) * P, :].rearrange(
                        "(t p) f -> p t f", p=P
                    ),
                )
                for t, ps in _emit_agg(g, msg, loc, sC, psC, outc, b2_sb, "hps"):
                    ob = oC.tile([P, outc], F32, tag="ob")
                    nc.vector.tensor_scalar(
                        out=ob[:, :],
                        in0=ps[:, :],
                        scalar1=dv_sb[:, t : t + 1],
                        scalar2=None,
                        op0=mybir.AluOpType.mult,
                    )
                    nc.sync.dma_start(out_d[t * P : (t + 1) * P, :], ob[:, :])

    st.close()
    nc.compile()
    return nc


def _make_in_maps(plan, x, W1, b1, W2, b2):
    inc, hid, outc = plan.inc, plan.hid, plan.outc
    ncc, hcc = inc // P, hid // P
    w1t = np.ascontiguousarray(
        W1.reshape(ncc, P, hid).transpose(1, 0, 2).astype(np.float16)
    )
    w2t = np.ascontiguousarray(
        W2.reshape(hcc, P, outc).transpose(1, 0, 2).astype(np.float16)
    )
    b1r = np.ascontiguousarray(b1.astype(np.float16).reshape(1, hid))
    b2r = np.ascontiguousarray(b2.astype(np.float16).reshape(1, outc))
    iota = np.ascontiguousarray(
        np.tile(np.arange(P, dtype=np.float32), (P, 1))
    )
    ident = np.ascontiguousarray(np.eye(P, dtype=np.float16))
    in_maps = []
    for c in range(CORES):
        xs = x[c * plan.shard : (c + 1) * plan.shard].astype(np.float32)
        xs = np.pad(xs, ((0, plan.shard_pad - plan.shard), (0, 0)))
        xt = xs.reshape(plan.tiles, P, ncc, P).transpose(0, 3, 2, 1)
        in_maps.append(
            {
                "xt": np.ascontiguousarray(xt.astype(np.float16)),
                "w1t": w1t,
                "w2t": w2t,
                "b1r": b1r,
                "b2r": b2r,
                "iota": iota,
                "ident": ident,
                "idx": plan.core_idx[c],
                "dstrel": plan.core_drel[c],
                "dinvc": plan.core_dinvc[c],
                "sqd": plan.core_sqd[c],
            }
        )
    return in_maps


_CACHE = {}


def _get_built(x, edge_index, W1, b1, W2, b2):
    n_nodes, in_c = x.shape
    hid = W1.shape[1]
    out_c = W2.shape[1]
    key = (n_nodes, in_c, hid, out_c, hash(edge_index.tobytes()))
    if key not in _CACHE:
        src = np.asarray(edge_index[0], np.int64)
        dst = np.asarray(edge_index[1], np.int64)
        plan = Plan(n_nodes, src, dst, in_c, hid, out_c)
        nc = _build_nc(plan)
        _CACHE[key] = (plan, nc)
    return _CACHE[key]


def run(x, edge_index, W1, b1, W2, b2, trace=False, **spmd_kwargs):
    plan, nc = _get_built(x, edge_index, W1, b1, W2, b2)
    in_maps = _make_in_maps(plan, x, W1, b1, W2, b2)
    res = run_bass_kernel_spmd(
        nc, in_maps, core_ids=list(range(CORES)), trace=trace, **spmd_kwargs
    )
    out = np.concatenate(
        [res.results[c]["out"][: plan.shard] for c in range(CORES)], axis=0
    ).astype(np.float32)
    return out, res


def kernel(**inputs):
    x = np.asarray(inputs["x"], np.float32)
    edge_index = np.asarray(inputs["edge_index"])
    W1 = np.asarray(inputs["W1"], np.float32)
    b1 = np.asarray(inputs["b1"], np.float32)
    W2 = np.asarray(inputs["W2"], np.float32)
    b2 = np.asarray(inputs["b2"], np.float32)
    out, _ = run(x, edge_index, W1, b1, W2, b2)
    return out



# revision 14
# speedup vs baseline: 1.1558x; 1.0309x over previous
"""Trainium2 Bass kernel for a 2-layer GCN encoder (PyG GCNConv semantics).

Strategy (8 NeuronCores, SPMD):
  * Nodes sharded across 8 cores (6250/core); edges partitioned by
    destination shard; weights replicated.
  * Dense layer projections on the local shard; h-tables exchanged with
    chunked AllGathers (two contiguous Shared tables A/B per layer, split
    at local row 3072) that overlap the producing compute; per-shard
    aggregation via batched gather DMAs (dma_gather) + one-hot
    selection-matrix matmuls on the TensorEngine.
  * Gather ordering against the AllGather chunks is handled by Tile's
    dependency tracking (collectives are issued inside the TileContext).
  * Gather descriptor generation is spread over all 4 SWDGE queues so all
    four Q7 DSP pairs generate descriptors concurrently.
  * Edge chunks are packed per (group, A/B section) with tiles sharing
    boundary chunks: a chunk overlapping two destination tiles gets one
    selection plane per (chunk, tile) pair (padding ~3% instead of ~17%).
  * Self-loops are NOT gathered: their contribution dinv[d]*h[d] is added
    with one identity matmul per tile over the local h' rows.
  * The symmetric norm dinv[src]*dinv[dst] is factored: dinv[src] is
    pre-scaled into the gathered tables (h' = D^-1/2 h), dinv[dst] is a
    per-partition post-scale on the aggregated PSUM.
  * Biases are folded in as rank-1 matmuls (sqrt(deg)[n] x b[f]) inside
    the accumulation group, so out = dinv * (R + sqd*b) = dinv*R + b.
  * Tables in fp16 (accumulation in fp32 PSUM).

All preprocessing (degrees, edge sorting/packing, int16 gather index
tables) happens on the host in numpy inside kernel().
"""

import contextlib
import os
import sys

import numpy as np

for _p in ("/opt/trn_rl_repo", "/root/.axon_site/_ro/trn_rl_repo"):
    if os.path.isdir(_p) and _p not in sys.path:
        sys.path.insert(0, _p)

import concourse.bacc as bacc
import concourse.mybir as mybir
import concourse.tile as tile
from concourse.bass_utils import run_bass_kernel_spmd
from concourse.library_config import mlp as _mlp_lib

P = 128
CORES = 8
GRP = 4  # destination-node tiles per gather group
GCAP = 1024  # max rows per dma_gather call (single-packet ceiling)
NQ = 4  # SWDGE queues: gather desc-gen runs on DSP pair (2q, 2q+1)

F8 = mybir.dt.float8e4
F16 = mybir.dt.float16
F32 = mybir.dt.float32
I16 = mybir.dt.int16


def _cdiv(a, b):
    return -(-a // b)


class Plan:
    """Static (cross-core identical) schedule + per-core host arrays.

    Sections: A = sources with local row < split_r (gather table A =
    AllGather chunk A, rows c*split_r + r), B = the rest (table B, rows
    c*(shard_pad-split_r) + r - split_r).  Both tables fit int16.
    Chunk schedule: per group of GRP destination tiles and per section,
    the member tiles' edge runs are packed back-to-back; chunk j gets one
    selection plane per destination tile any core's run overlaps.  Rows
    are padded (gathering table row 0 harmlessly: the one-hot has
    drel=-1 there) only at the section tail, up to the max core's count.
    Self-loops are excluded (identity matmul adds them on-device).
    """

    def __init__(self, n_nodes, edge_src, edge_dst, in_c, hid, out_c):
        assert n_nodes % CORES == 0
        self.n = n_nodes
        self.inc, self.hid, self.outc = in_c, hid, out_c
        self.shard = n_nodes // CORES
        self.tiles = _cdiv(self.shard, P)
        self.shard_pad = self.tiles * P
        self.npad = self.shard_pad * CORES
        self.split_t = 20  # tiles in chunk/table A (GRP-aligned; ships early)
        self.split_r = self.split_t * P
        self.b_rows = self.shard_pad - self.split_r  # per-core rows, table B
        assert CORES * self.split_r < 32768
        assert CORES * self.b_rows < 32768

        deg = np.bincount(edge_dst, minlength=n_nodes).astype(np.float64) + 1.0
        self.dinv_full = 1.0 / np.sqrt(deg)
        dinv = self.dinv_full

        shard = self.shard
        tiles = self.tiles
        core_of = edge_dst // shard
        percore = []  # (idx16, t, drel, isa)
        cnt = np.zeros((CORES, tiles, 2), np.int64)  # [core, tile, sec]
        for c in range(CORES):
            m = core_of == c
            s = edge_src[m]
            d = edge_dst[m]
            sc = s // shard
            r = s % shard
            isa = r < self.split_r
            idx16 = np.where(
                isa, sc * self.split_r + r, sc * self.b_rows + r - self.split_r
            )
            dloc = d - c * shard
            t = dloc // P
            drel = dloc % P
            percore.append((idx16, t, drel, isa))
            for sec in (0, 1):
                msec = isa if sec == 0 else ~isa
                cnt[c, :, sec] = np.bincount(t[msec], minlength=tiles)

        # group schedule
        self.groups = []
        chunk_base = 0  # running chunk count (= msg cols / gather rows /128)
        plane_base = 0  # running plane count (= dr_sb cols)
        idxcol_base = 0
        for g0 in range(0, tiles, GRP):
            gt = list(range(g0, min(g0 + GRP, tiles)))
            secs = []
            for sec in (0, 1):
                ccnt = cnt[:, gt, sec]  # [core, tiles-in-group]
                rows = ccnt.sum(axis=1)  # per-core section rows
                nch = int(_cdiv(int(rows.max()), P)) if rows.max() else 0
                offs = np.zeros((CORES, len(gt) + 1), np.int64)
                offs[:, 1:] = np.cumsum(ccnt, axis=1)
                tiles_of = [[] for _ in range(nch)]
                for j in range(nch):
                    lo, hi = j * P, (j + 1) * P
                    for k, t in enumerate(gt):
                        if ((offs[:, k] < hi) & (offs[:, k + 1] > lo)).any():
                            tiles_of[j].append(k)
                plane_list = []  # (k local-tile, j chunk)
                tile_planes = {k: [] for k in range(len(gt))}
                for k in range(len(gt)):
                    for j in range(nch):
                        if k in tiles_of[j]:
                            tile_planes[k].append((len(plane_list), j))
                            plane_list.append((k, j))
                secs.append(
                    dict(
                        nch=nch,
                        offs=offs,
                        plane_list=plane_list,
                        tile_planes=tile_planes,
                        chunk_off=0,
                        plane_off=0,
                    )
                )
            secs[1]["chunk_off"] = secs[0]["nch"]
            secs[1]["plane_off"] = len(secs[0]["plane_list"])
            nch_g = secs[0]["nch"] + secs[1]["nch"]
            npl_g = len(secs[0]["plane_list"]) + len(secs[1]["plane_list"])
            self.groups.append(
                dict(
                    tiles=gt,
                    secs=secs,
                    nch=nch_g,
                    nplanes=npl_g,
                    chunk_base=chunk_base,
                    plane_base=plane_base,
                    idxcol_base=idxcol_base,
                )
            )
            chunk_base += nch_g
            plane_base += npl_g
            idxcol_base += nch_g * P // 16
        self.tot_chunks = chunk_base
        self.tot_planes = plane_base
        self.tot_idxcols = idxcol_base

        # per-core flat arrays in schedule order
        self.core_idx = []
        self.core_drel = []
        self.core_dinvc = []
        self.core_sqd = []
        for c in range(CORES):
            idx16, t_arr, drel, isa = percore[c]
            idx_flat = np.zeros(self.tot_chunks * P, np.int32)
            drel_planes = np.full((self.tot_planes, P), -1.0, np.float32)
            for g in self.groups:
                gt = g["tiles"]
                for sec in (0, 1):
                    S = g["secs"][sec]
                    if not S["nch"]:
                        continue
                    msec = isa if sec == 0 else ~isa
                    base_row = (g["chunk_base"] + S["chunk_off"]) * P
                    offs = S["offs"][c]
                    for k, t in enumerate(gt):
                        sel = (t_arr == t) & msec
                        kcnt = int(sel.sum())
                        assert kcnt == offs[k + 1] - offs[k]
                        idx_flat[
                            base_row + offs[k] : base_row + offs[k] + kcnt
                        ] = idx16[sel]
                        for pl, j in S["tile_planes"][k]:
                            lo = max(int(offs[k]), j * P)
                            hi = min(int(offs[k + 1]), (j + 1) * P)
                            if hi <= lo:
                                continue
                            rows = np.arange(lo, hi)
                            drel_planes[
                                g["plane_base"] + S["plane_off"] + pl,
                                rows - j * P,
                            ] = drel[sel][lo - int(offs[k]) : hi - int(offs[k])]
            # wrap idx into the dma_gather SBUF layout [128, cols]: per
            # call, idx i lives at [p, i // 16] for p % 16 == i % 16,
            # replicated 8x over partitions.  Calls are <=GCAP rows.
            blocks = []
            for g in self.groups:
                a = g["chunk_base"] * P
                for sec in (0, 1):
                    n = g["secs"][sec]["nch"] * P
                    for off in range(0, n, GCAP):
                        nn = min(GCAP, n - off)
                        v = idx_flat[a + off : a + off + nn].reshape(-1, 16).T
                        blocks.append(np.tile(v, (8, 1)))
                    a += n
            idx_sb = (
                np.concatenate(blocks, axis=1).astype(np.int16)
                if blocks
                else np.zeros((P, 0), np.int16)
            )
            assert idx_sb.shape == (P, self.tot_idxcols), idx_sb.shape
            self.core_idx.append(np.ascontiguousarray(idx_sb))
            self.core_drel.append(np.ascontiguousarray(drel_planes.T))
            # per-node scale planes for this shard (pad nodes: dinv=1, sqd=0)
            dshard = np.ones(self.shard_pad, np.float64)
            dshard[:shard] = dinv[c * shard : (c + 1) * shard]
            dc = dshard.reshape(self.tiles, P).T.astype(np.float32)
            self.core_dinvc.append(np.ascontiguousarray(dc))
            sq = np.zeros(self.shard_pad, np.float64)
            sq[:shard] = 1.0 / dinv[c * shard : (c + 1) * shard]
            self.core_sqd.append(sq.astype(np.float16).reshape(1, self.shard_pad))


def _build_nc(plan):
    inc, hid, outc = plan.inc, plan.hid, plan.outc
    ncc = inc // P  # contraction chunks for layer-1 projection
    hcc = hid // P  # contraction chunks for layer-2 projection
    tiles = plan.tiles
    spl_r = plan.split_r
    spl_t = plan.split_t

    nc = bacc.Bacc("TRN2", num_devices=CORES, num_swdge_queues=NQ)

    xt_d = nc.dram_tensor("xt", [tiles, P, ncc, P], F16, kind="ExternalInput")
    w1_d = nc.dram_tensor("w1t", [P, ncc, hid], F16, kind="ExternalInput")
    w2_d = nc.dram_tensor("w2t", [P, hcc, outc], F16, kind="ExternalInput")
    b1_d = nc.dram_tensor("b1r", [1, hid], F16, kind="ExternalInput")
    b2_d = nc.dram_tensor("b2r", [1, outc], F16, kind="ExternalInput")
    io_d = nc.dram_tensor("iota", [P, P], F32, kind="ExternalInput")
    id_d = nc.dram_tensor("ident", [P, P], F16, kind="ExternalInput")
    ix_d = nc.dram_tensor("idx", [P, plan.tot_idxcols], I16, kind="ExternalInput")
    dr_d = nc.dram_tensor("dstrel", [P, plan.tot_planes], F32, kind="ExternalInput")
    dv_d = nc.dram_tensor("dinvc", [P, tiles], F32, kind="ExternalInput")
    sq_d = nc.dram_tensor("sqd", [1, plan.shard_pad], F16, kind="ExternalInput")
    out_d = nc.dram_tensor("out", [plan.shard_pad, outc], F32, kind="ExternalOutput")

    h1_sh = nc.dram_tensor("h1_shard", [plan.shard_pad, hid], F8)
    h1_fa = nc.dram_tensor("h1_fa", [CORES * spl_r, hid], F8, addr_space="Shared")
    h1_fb = nc.dram_tensor("h1_fb", [CORES * plan.b_rows, hid], F8, addr_space="Shared")
    h2_sh = nc.dram_tensor("h2_shard", [plan.shard_pad, outc], F16)
    h2_fa = nc.dram_tensor("h2_fa", [CORES * spl_r, outc], F16, addr_space="Shared")
    h2_fb = nc.dram_tensor("h2_fb", [CORES * plan.b_rows, outc], F16, addr_space="Shared")

    st = contextlib.ExitStack()
    idx_sb = st.enter_context(nc.sbuf_tensor("idx_sb", [P, plan.tot_idxcols], I16))
    dr_sb = st.enter_context(nc.sbuf_tensor("dr_sb", [P, plan.tot_planes], F32))
    io_sb = st.enter_context(nc.sbuf_tensor("io_sb", [P, P], F32))
    id_sb = st.enter_context(nc.sbuf_tensor("id_sb", [P, P], F16))
    w1_sb = st.enter_context(nc.sbuf_tensor("w1_sb", [P, ncc, hid], F16))
    w2_sb = st.enter_context(nc.sbuf_tensor("w2_sb", [P, hcc, outc], F16))
    b1_sb = st.enter_context(nc.sbuf_tensor("b1_sb", [1, hid], F16))
    b2_sb = st.enter_context(nc.sbuf_tensor("b2_sb", [1, outc], F16))
    dv_sb = st.enter_context(nc.sbuf_tensor("dv_sb", [P, tiles], F32))
    sq_sb = st.enter_context(nc.sbuf_tensor("sq_sb", [1, plan.shard_pad], F16))

    nc.gpsimd.load_library(_mlp_lib)

    # one gpsimd register per distinct gather count
    _regs = {}

    def _nreg(v):
        if v not in _regs:
            _regs[v] = nc.gpsimd.to_reg(v)
        return _regs[v]

    for g in plan.groups:
        for sec in (0, 1):
            n = g["secs"][sec]["nch"] * P
            for off in range(0, n, GCAP):
                _nreg(min(GCAP, n - off))

    _qctr = [0]

    def _emit_gather(msg, ch0, nch, table, ic0, elem):
        """Gather nch*P rows into msg[:, ch0:ch0+nch, :] in <=GCAP pieces.

        Calls round-robin over the SWDGE queues so descriptor generation
        runs on all four Q7 DSP pairs concurrently."""
        n = nch * P
        ic = ic0
        for off in range(0, n, GCAP):
            nn = min(GCAP, n - off)
            nc.gpsimd.dma_gather(
                msg[:, ch0 + off // P : ch0 + (off + nn) // P, :],
                table,
                idx_sb[:, ic : ic + nn // 16],
                nn,
                _nreg(nn),
                elem,
                queue_num=_qctr[0] % NQ,
            )
            _qctr[0] += 1
            ic += nn // 16
        return ic

    def _emit_group_gathers(g, msg, ta, tb, elem):
        ic = g["idxcol_base"]
        an = g["secs"][0]["nch"]
        bn = g["secs"][1]["nch"]
        if an:
            ic = _emit_gather(msg, 0, an, ta, ic, elem)
        if bn:
            _emit_gather(msg, an, bn, tb, ic, elem)

    def _emit_agg(g, msg, loc, sB, psPool, width, b_sb, ptag):
        """Per-tile one-hot aggregation matmuls for group g (+ self-loop
        identity matmul over the local h' rows + rank-1 bias).

        Returns list of (tile_global, psum_tile)."""
        out = []
        for k, t in enumerate(g["tiles"]):
            planes = []  # (plane_col absolute, chunk col within group msg)
            for sec in (0, 1):
                S = g["secs"][sec]
                for pl, j in S["tile_planes"][k]:
                    planes.append(
                        (
                            g["plane_base"] + S["plane_off"] + pl,
                            S["chunk_off"] + j,
                        )
                    )
            npl = len(planes)
            Stile = sB.tile([P, max(npl, 1), P], F16, tag="S")
            pos = 0
            for sec in (0, 1):
                S = g["secs"][sec]
                tp = S["tile_planes"][k]
                if not tp:
                    continue
                c0 = g["plane_base"] + S["plane_off"] + tp[0][0]
                n = len(tp)
                assert tp[-1][0] - tp[0][0] == n - 1  # contiguous
                nc.vector.tensor_tensor(
                    out=Stile[:, pos : pos + n, :],
                    in0=io_sb[:, None, :].to_broadcast([P, n, P]),
                    in1=dr_sb[:, c0 : c0 + n][:, :, None].to_broadcast(
                        [P, n, P]
                    ),
                    op=mybir.AluOpType.is_equal,
                )
                pos += n
            ps = psPool.tile([P, width], F32, tag=ptag)
            for i, (_pl, ch) in enumerate(planes):
                nc.tensor.matmul(
                    ps[:, :],
                    lhsT=Stile[:, i, :],
                    rhs=msg[:, ch, :],
                    start=(i == 0),
                    stop=False,
                )
            # self-loop: += dinv[d]*h[d] (local h' rows, identity matmul)
            nc.tensor.matmul(
                ps[:, :],
                lhsT=id_sb[:, :],
                rhs=loc[:, k, :],
                start=(npl == 0),
                stop=False,
            )
            # rank-1 bias: += sqrt(deg)[n] * b[f]
            nc.tensor.matmul(
                ps[:, :],
                lhsT=sq_sb[0:1, t * P : (t + 1) * P],
                rhs=b_sb[0:1, :],
                start=False,
                stop=True,
            )
            out.append((t, ps))
        return out

    # ctx0: resident loads (own context so its exit barrier orders them
    # before every later consumer on every engine).
    with tile.TileContext(nc):
        nc.sync.dma_start(idx_sb[:, :], ix_d[:, :])
        nc.sync.dma_start(dr_sb[:, :], dr_d[:, :])
        nc.sync.dma_start(io_sb[:, :], io_d[:, :])
        nc.sync.dma_start(id_sb[:, :], id_d[:, :])
        nc.sync.dma_start(w1_sb[:, :, :], w1_d[:, :, :])
        nc.sync.dma_start(w2_sb[:, :, :], w2_d[:, :, :])
        nc.sync.dma_start(b1_sb[:, :], b1_d[:, :])
        nc.sync.dma_start(b2_sb[:, :], b2_d[:, :])
        nc.sync.dma_start(dv_sb[:, :], dv_d[:, :])
        nc.sync.dma_start(sq_sb[:, :], sq_d[:, :])

    # Fused context: dense projection, chunked AllGathers, both
    # aggregation layers all in one TileContext so the scheduler overlaps
    # collectives and gathers with compute across phase boundaries.
    with tile.TileContext(nc) as tc:
      with (
        tc.tile_pool(name="pA", bufs=2) as pA,
        tc.tile_pool(name="msgB", bufs=3) as msgB,
        tc.tile_pool(name="locB", bufs=2) as locB,
        tc.tile_pool(name="sB", bufs=2) as sB,
        tc.tile_pool(name="zB", bufs=2) as zB,
        tc.tile_pool(name="h2B", bufs=3) as h2B,
        tc.tile_pool(name="psB", bufs=3, space="PSUM") as psB,
        tc.tile_pool(name="psT", bufs=2, space="PSUM") as psT,
        tc.tile_pool(name="psH", bufs=3, space="PSUM") as psH,
        tc.tile_pool(name="msgC", bufs=3) as msgC,
        tc.tile_pool(name="locC", bufs=2) as locC,
        tc.tile_pool(name="sC", bufs=3) as sC,
        tc.tile_pool(name="oC", bufs=3) as oC,
      ):
        psA = psB   # same [P, hid] f32 tiles; stay within 8 PSUM banks
        psC = psH   # same [P, outc] f32 tiles
        # layer-1 dense projection h1' = D^-1/2 (x @ W1), 2 tiles/step;
        # each half's h1 rows AllGather as soon as they are written.
        for hi, (s0, s1) in enumerate([(0, spl_t), (spl_t, tiles)]):
                for t0 in range(s0, s1, 4):
                    nt = min(4, s1 - t0)
                    xa = pA.tile([P, nt, ncc, P], F16, tag="xa")
                    nc.sync.dma_start(xa[:, :, :, :], xt_d[t0 : t0 + nt, :, :, :].rearrange("t p c n -> p t c n"))
                    h1t = pA.tile([P, nt, hid], F8, tag="h1t")
                    for k in range(nt):
                        ps = psA.tile([P, hid], F32, tag="psAgg")
                        for cc in range(ncc):
                            nc.tensor.matmul(
                                ps[:, :],
                                lhsT=xa[:, k, cc, :],
                                rhs=w1_sb[:, cc, :],
                                start=(cc == 0),
                                stop=(cc == ncc - 1),
                            )
                        nc.scalar.activation(
                            h1t[:, k, :],
                            ps[:, :],
                            mybir.ActivationFunctionType.Copy,
                            scale=dv_sb[:, t0 + k : t0 + k + 1],
                        )
                    nc.sync.dma_start(
                        h1_sh[t0 * P : (t0 + nt) * P, :].rearrange(
                            "(t p) f -> p t f", p=P
                        ),
                        h1t[:, :, :],
                    )
                nc.gpsimd.collective_compute(
                    "AllGather",
                    mybir.AluOpType.bypass,
                    replica_groups=[list(range(CORES))],
                    ins=[h1_sh[s0 * P : s1 * P, :].bitcast(F32)],
                    outs=[(h1_fa if hi == 0 else h1_fb)[:, :].bitcast(F32)],
                )

        # layer-1 aggregate + relu, then layer-2 dense projection; the
        # first part's h2 rows AllGather while the second part computes.
        gsplit = spl_t // GRP
        assert plan.groups[gsplit]["tiles"][0] == spl_t
        for hi, (gs0, gs1) in enumerate([(0, gsplit), (gsplit, len(plan.groups))]):
            for g in plan.groups[gs0:gs1]:
                nt = len(g["tiles"])
                t0 = g["tiles"][0]
                msg = msgB.tile([P, g["nch"], hid], F8, tag="msg")
                _emit_group_gathers(g, msg, h1_fa[:, :], h1_fb[:, :], hid)
                loc = locB.tile([P, nt, hid], F8, tag="loc")
                nc.sync.dma_start(
                    loc[:, :, :],
                    h1_sh[t0 * P : (t0 + nt) * P, :].rearrange(
                        "(t p) f -> p t f", p=P
                    ),
                )
                for t, ps in _emit_agg(g, msg, loc, sB, psB, hid, b1_sb, "psAgg"):
                    # z1 = relu(dinv[n] * ps)  [node, f] fp16
                    z1 = zB.tile([P, hid], F16, tag="z1")
                    nc.vector.tensor_scalar(
                        out=z1[:, :],
                        in0=ps[:, :],
                        scalar1=dv_sb[:, t : t + 1],
                        scalar2=0.0,
                        op0=mybir.AluOpType.mult,
                        op1=mybir.AluOpType.max,
                    )
                    # transpose z1 -> z1T for the layer-2 contraction
                    zt_ps = psT.tile([P, hcc, P], F16, tag="ztps")
                    for h in range(hcc):
                        nc.tensor.transpose(
                            zt_ps[:, h, :],
                            z1[:, h * P : (h + 1) * P],
                            id_sb[:, :],
                        )
                    zt = zB.tile([P, hcc, P], F16, tag="zt")
                    nc.scalar.activation(
                        zt[:, :, :],
                        zt_ps[:, :, :],
                        mybir.ActivationFunctionType.Copy,
                    )
                    hps = psH.tile([P, outc], F32, tag="hps")
                    for cc in range(hcc):
                        nc.tensor.matmul(
                            hps[:, :],
                            lhsT=zt[:, cc, :],
                            rhs=w2_sb[:, cc, :],
                            start=(cc == 0),
                            stop=(cc == hcc - 1),
                        )
                    # h2' = dinv[n] * (z1 @ W2)
                    h2t = h2B.tile([P, outc], F16, tag="h2t")
                    nc.scalar.activation(
                        h2t[:, :],
                        hps[:, :],
                        mybir.ActivationFunctionType.Copy,
                        scale=dv_sb[:, t : t + 1],
                    )
                    nc.sync.dma_start(h2_sh[t * P : (t + 1) * P, :], h2t[:, :])
            r0 = plan.groups[gs0]["tiles"][0] * P
            r1 = (plan.groups[gs1 - 1]["tiles"][-1] + 1) * P
            nc.gpsimd.collective_compute(
                "AllGather",
                mybir.AluOpType.bypass,
                replica_groups=[list(range(CORES))],
                ins=[h2_sh[r0:r1, :].bitcast(F32)],
                outs=[(h2_fa if hi == 0 else h2_fb)[:, :].bitcast(F32)],
            )

        # layer-2 aggregate + bias -> output
        if True:
            for g in plan.groups:
                nt = len(g["tiles"])
                t0 = g["tiles"][0]
                msg = msgC.tile([P, g["nch"], outc], F16, tag="msg2")
                _emit_group_gathers(g, msg, h2_fa[:, :], h2_fb[:, :], outc)
                loc = locC.tile([P, nt, outc], F16, tag="loc2")
                nc.sync.dma_start(
                    loc[:, :, :],
                    h2_sh[t0 * P : (t0 + nt) * P, :].rearrange(
                        "(t p) f -> p t f", p=P
                    ),
                )
                for t, ps in _emit_agg(g, msg, loc, sC, psC, outc, b2_sb, "hps"):
                    ob = oC.tile([P, outc], F32, tag="ob")
                    nc.vector.tensor_scalar(
                        out=ob[:, :],
                        in0=ps[:, :],
                        scalar1=dv_sb[:, t : t + 1],
                        scalar2=None,
                        op0=mybir.AluOpType.mult,
                    )
                    nc.sync.dma_start(out_d[t * P : (t + 1) * P, :], ob[:, :])

    st.close()
    nc.compile()
    return nc


def _make_in_maps(plan, x, W1, b1, W2, b2):
    inc, hid, outc = plan.inc, plan.hid, plan.outc
    ncc, hcc = inc // P, hid // P
    w1t = np.ascontiguousarray(
        W1.reshape(ncc, P, hid).transpose(1, 0, 2).astype(np.float16)
    )
    w2t = np.ascontiguousarray(
        W2.reshape(hcc, P, outc).transpose(1, 0, 2).astype(np.float16)
    )
    b1r = np.ascontiguousarray(b1.astype(np.float16).reshape(1, hid))
    b2r = np.ascontiguousarray(b2.astype(np.float16).reshape(1, outc))
    iota = np.ascontiguousarray(
        np.tile(np.arange(P, dtype=np.float32), (P, 1))
    )
    ident = np.ascontiguousarray(np.eye(P, dtype=np.float16))
    in_maps = []
    for c in range(CORES):
        xs = x[c * plan.shard : (c + 1) * plan.shard].astype(np.float32)
        xs = np.pad(xs, ((0, plan.shard_pad - plan.shard), (0, 0)))
        xt = xs.reshape(plan.tiles, P, ncc, P).transpose(0, 3, 2, 1)
        in_maps.append(
            {
                "xt": np.ascontiguousarray(xt.astype(np.float16)),
                "w1t": w1t,
                "w2t": w2t,
                "b1r": b1r,
                "b2r": b2r,
                "iota": iota,
                "ident": ident,
                "idx": plan.core_idx[c],
                "dstrel": plan.core_drel[c],
                "dinvc": plan.core_dinvc[c],
                "sqd": plan.core_sqd[c],
            }
        )
    return in_maps


_CACHE = {}


def _get_built(x, edge_index, W1, b1, W2, b2):
    n_nodes, in_c = x.shape
    hid = W1.shape[1]
    out_c = W2.shape[1]
    key = (n_nodes, in_c, hid, out_c, hash(edge_index.tobytes()))
    if key not in _CACHE:
        src = np.asarray(edge_index[0], np.int64)
        dst = np.asarray(edge_index[1], np.int64)
        plan = Plan(n_nodes, src, dst, in_c, hid, out_c)
        nc = _build_nc(plan)
        _CACHE[key] = (plan, nc)
    return _CACHE[key]


def run(x, edge_index, W1, b1, W2, b2, trace=False, **spmd_kwargs):
    plan, nc = _get_built(x, edge_index, W1, b1, W2, b2)
    in_maps = _make_in_maps(plan, x, W1, b1, W2, b2)
    res = run_bass_kernel_spmd(
        nc, in_maps, core_ids=list(range(CORES)), trace=trace, **spmd_kwargs
    )
    out = np.concatenate(
        [res.results[c]["out"][: plan.shard] for c in range(CORES)], axis=0
    ).astype(np.float32)
    return out, res


def kernel(**inputs):
    x = np.asarray(inputs["x"], np.float32)
    edge_index = np.asarray(inputs["edge_index"])
    W1 = np.asarray(inputs["W1"], np.float32)
    b1 = np.asarray(inputs["b1"], np.float32)
    W2 = np.asarray(inputs["W2"], np.float32)
    b2 = np.asarray(inputs["b2"], np.float32)
    out, _ = run(x, edge_index, W1, b1, W2, b2)
    return out



# revision 15
# speedup vs baseline: 1.1634x; 1.0066x over previous
"""Trainium2 Bass kernel for a 2-layer GCN encoder (PyG GCNConv semantics).

Strategy (8 NeuronCores, SPMD):
  * Nodes sharded across 8 cores (6250/core); edges partitioned by
    destination shard; weights replicated.
  * Dense layer projections on the local shard; h-tables exchanged with
    chunked AllGathers (two contiguous Shared tables A/B per layer, split
    at local row 3072) that overlap the producing compute; per-shard
    aggregation via batched gather DMAs (dma_gather) + one-hot
    selection-matrix matmuls on the TensorEngine.
  * Gather ordering against the AllGather chunks is handled by Tile's
    dependency tracking (collectives are issued inside the TileContext).
  * Gather descriptor generation is spread over all 4 SWDGE queues so all
    four Q7 DSP pairs generate descriptors concurrently.
  * Edge chunks are packed per (group, A/B section) with tiles sharing
    boundary chunks: a chunk overlapping two destination tiles gets one
    selection plane per (chunk, tile) pair (padding ~3% instead of ~17%).
  * Self-loops are NOT gathered: their contribution dinv[d]*h[d] is added
    with one identity matmul per tile over the local h' rows.
  * The symmetric norm dinv[src]*dinv[dst] is factored: dinv[src] is
    pre-scaled into the gathered tables (h' = D^-1/2 h), dinv[dst] is a
    per-partition post-scale on the aggregated PSUM.
  * Biases are folded in as rank-1 matmuls (sqrt(deg)[n] x b[f]) inside
    the accumulation group, so out = dinv * (R + sqd*b) = dinv*R + b.
  * Tables in fp16 (accumulation in fp32 PSUM).

All preprocessing (degrees, edge sorting/packing, int16 gather index
tables) happens on the host in numpy inside kernel().
"""

import contextlib
import os
import sys

import numpy as np

for _p in ("/opt/trn_rl_repo", "/root/.axon_site/_ro/trn_rl_repo"):
    if os.path.isdir(_p) and _p not in sys.path:
        sys.path.insert(0, _p)

import concourse.bacc as bacc
import concourse.mybir as mybir
import concourse.tile as tile
from concourse.bass_utils import run_bass_kernel_spmd
from concourse.library_config import mlp as _mlp_lib

P = 128
CORES = 8
GRP = 4  # destination-node tiles per gather group
GCAP = 1024  # max rows per dma_gather call (single-packet ceiling)
NQ = 4  # SWDGE queues: gather desc-gen runs on DSP pair (2q, 2q+1)

F8 = mybir.dt.float8e4
F16 = mybir.dt.float16
F32 = mybir.dt.float32
I16 = mybir.dt.int16


def _cdiv(a, b):
    return -(-a // b)


class Plan:
    """Static (cross-core identical) schedule + per-core host arrays.

    Sections: A = sources with local row < split_r (gather table A =
    AllGather chunk A, rows c*split_r + r), B = the rest (table B, rows
    c*(shard_pad-split_r) + r - split_r).  Both tables fit int16.
    Chunk schedule: per group of GRP destination tiles and per section,
    the member tiles' edge runs are packed back-to-back; chunk j gets one
    selection plane per destination tile any core's run overlaps.  Rows
    are padded (gathering table row 0 harmlessly: the one-hot has
    drel=-1 there) only at the section tail, up to the max core's count.
    Self-loops are excluded (identity matmul adds them on-device).
    """

    def __init__(self, n_nodes, edge_src, edge_dst, in_c, hid, out_c):
        assert n_nodes % CORES == 0
        self.n = n_nodes
        self.inc, self.hid, self.outc = in_c, hid, out_c
        self.shard = n_nodes // CORES
        self.tiles = _cdiv(self.shard, P)
        self.shard_pad = self.tiles * P
        self.npad = self.shard_pad * CORES
        self.split_t = 20  # tiles in chunk/table A (GRP-aligned; ships early)
        self.split_r = self.split_t * P
        self.b_rows = self.shard_pad - self.split_r  # per-core rows, table B
        assert CORES * self.split_r < 32768
        assert CORES * self.b_rows < 32768

        deg = np.bincount(edge_dst, minlength=n_nodes).astype(np.float64) + 1.0
        self.dinv_full = 1.0 / np.sqrt(deg)
        dinv = self.dinv_full

        shard = self.shard
        tiles = self.tiles
        core_of = edge_dst // shard
        percore = []  # (idx16, t, drel, isa)
        cnt = np.zeros((CORES, tiles, 2), np.int64)  # [core, tile, sec]
        for c in range(CORES):
            m = core_of == c
            s = edge_src[m]
            d = edge_dst[m]
            sc = s // shard
            r = s % shard
            isa = r < self.split_r
            idx16 = np.where(
                isa, sc * self.split_r + r, sc * self.b_rows + r - self.split_r
            )
            dloc = d - c * shard
            t = dloc // P
            drel = dloc % P
            percore.append((idx16, t, drel, isa))
            for sec in (0, 1):
                msec = isa if sec == 0 else ~isa
                cnt[c, :, sec] = np.bincount(t[msec], minlength=tiles)

        # group schedule
        self.groups = []
        chunk_base = 0  # running chunk count (= msg cols / gather rows /128)
        plane_base = 0  # running plane count (= dr_sb cols)
        idxcol_base = 0
        for g0 in range(0, tiles, GRP):
            gt = list(range(g0, min(g0 + GRP, tiles)))
            secs = []
            for sec in (0, 1):
                ccnt = cnt[:, gt, sec]  # [core, tiles-in-group]
                rows = ccnt.sum(axis=1)  # per-core section rows
                nch = int(_cdiv(int(rows.max()), P)) if rows.max() else 0
                offs = np.zeros((CORES, len(gt) + 1), np.int64)
                offs[:, 1:] = np.cumsum(ccnt, axis=1)
                tiles_of = [[] for _ in range(nch)]
                for j in range(nch):
                    lo, hi = j * P, (j + 1) * P
                    for k, t in enumerate(gt):
                        if ((offs[:, k] < hi) & (offs[:, k + 1] > lo)).any():
                            tiles_of[j].append(k)
                plane_list = []  # (k local-tile, j chunk)
                tile_planes = {k: [] for k in range(len(gt))}
                for k in range(len(gt)):
                    for j in range(nch):
                        if k in tiles_of[j]:
                            tile_planes[k].append((len(plane_list), j))
                            plane_list.append((k, j))
                secs.append(
                    dict(
                        nch=nch,
                        offs=offs,
                        plane_list=plane_list,
                        tile_planes=tile_planes,
                        chunk_off=0,
                        plane_off=0,
                    )
                )
            secs[1]["chunk_off"] = secs[0]["nch"]
            secs[1]["plane_off"] = len(secs[0]["plane_list"])
            nch_g = secs[0]["nch"] + secs[1]["nch"]
            npl_g = len(secs[0]["plane_list"]) + len(secs[1]["plane_list"])
            self.groups.append(
                dict(
                    tiles=gt,
                    secs=secs,
                    nch=nch_g,
                    nplanes=npl_g,
                    chunk_base=chunk_base,
                    plane_base=plane_base,
                    idxcol_base=idxcol_base,
                )
            )
            chunk_base += nch_g
            plane_base += npl_g
            idxcol_base += nch_g * P // 16
        self.tot_chunks = chunk_base
        self.tot_planes = plane_base
        self.tot_idxcols = idxcol_base

        # per-core flat arrays in schedule order
        self.core_idx = []
        self.core_drel = []
        self.core_dinvc = []
        self.core_sqd = []
        for c in range(CORES):
            idx16, t_arr, drel, isa = percore[c]
            idx_flat = np.zeros(self.tot_chunks * P, np.int32)
            drel_planes = np.full((self.tot_planes, P), -1.0, np.float32)
            for g in self.groups:
                gt = g["tiles"]
                for sec in (0, 1):
                    S = g["secs"][sec]
                    if not S["nch"]:
                        continue
                    msec = isa if sec == 0 else ~isa
                    base_row = (g["chunk_base"] + S["chunk_off"]) * P
                    offs = S["offs"][c]
                    for k, t in enumerate(gt):
                        sel = (t_arr == t) & msec
                        kcnt = int(sel.sum())
                        assert kcnt == offs[k + 1] - offs[k]
                        idx_flat[
                            base_row + offs[k] : base_row + offs[k] + kcnt
                        ] = idx16[sel]
                        for pl, j in S["tile_planes"][k]:
                            lo = max(int(offs[k]), j * P)
                            hi = min(int(offs[k + 1]), (j + 1) * P)
                            if hi <= lo:
                                continue
                            rows = np.arange(lo, hi)
                            drel_planes[
                                g["plane_base"] + S["plane_off"] + pl,
                                rows - j * P,
                            ] = drel[sel][lo - int(offs[k]) : hi - int(offs[k])]
            # wrap idx into the dma_gather SBUF layout [128, cols]: per
            # call, idx i lives at [p, i // 16] for p % 16 == i % 16,
            # replicated 8x over partitions.  Calls are <=GCAP rows.
            blocks = []
            for g in self.groups:
                a = g["chunk_base"] * P
                for sec in (0, 1):
                    n = g["secs"][sec]["nch"] * P
                    for off in range(0, n, GCAP):
                        nn = min(GCAP, n - off)
                        v = idx_flat[a + off : a + off + nn].reshape(-1, 16).T
                        blocks.append(np.tile(v, (8, 1)))
                    a += n
            idx_sb = (
                np.concatenate(blocks, axis=1).astype(np.int16)
                if blocks
                else np.zeros((P, 0), np.int16)
            )
            assert idx_sb.shape == (P, self.tot_idxcols), idx_sb.shape
            self.core_idx.append(np.ascontiguousarray(idx_sb))
            self.core_drel.append(np.ascontiguousarray(drel_planes.T))
            # per-node scale planes for this shard (pad nodes: dinv=1, sqd=0)
            dshard = np.ones(self.shard_pad, np.float64)
            dshard[:shard] = dinv[c * shard : (c + 1) * shard]
            dc = dshard.reshape(self.tiles, P).T.astype(np.float32)
            self.core_dinvc.append(np.ascontiguousarray(dc))
            sq = np.zeros(self.shard_pad, np.float64)
            sq[:shard] = 1.0 / dinv[c * shard : (c + 1) * shard]
            self.core_sqd.append(sq.astype(np.float16).reshape(1, self.shard_pad))


def _build_nc(plan):
    inc, hid, outc = plan.inc, plan.hid, plan.outc
    ncc = inc // P  # contraction chunks for layer-1 projection
    hcc = hid // P  # contraction chunks for layer-2 projection
    tiles = plan.tiles
    spl_r = plan.split_r
    spl_t = plan.split_t

    nc = bacc.Bacc("TRN2", num_devices=CORES, num_swdge_queues=NQ)

    xt_d = nc.dram_tensor("xt", [tiles, P, ncc, P], F16, kind="ExternalInput")
    w1_d = nc.dram_tensor("w1t", [P, ncc, hid], F16, kind="ExternalInput")
    w2_d = nc.dram_tensor("w2t", [P, hcc, outc], F16, kind="ExternalInput")
    b1_d = nc.dram_tensor("b1r", [1, hid], F16, kind="ExternalInput")
    b2_d = nc.dram_tensor("b2r", [1, outc], F16, kind="ExternalInput")
    io_d = nc.dram_tensor("iota", [P, P], F32, kind="ExternalInput")
    id_d = nc.dram_tensor("ident", [P, P], F16, kind="ExternalInput")
    ix_d = nc.dram_tensor("idx", [P, plan.tot_idxcols], I16, kind="ExternalInput")
    dr_d = nc.dram_tensor("dstrel", [P, plan.tot_planes], F32, kind="ExternalInput")
    dv_d = nc.dram_tensor("dinvc", [P, tiles], F32, kind="ExternalInput")
    sq_d = nc.dram_tensor("sqd", [1, plan.shard_pad], F16, kind="ExternalInput")
    out_d = nc.dram_tensor("out", [plan.shard_pad, outc], F32, kind="ExternalOutput")

    h1_sh = nc.dram_tensor("h1_shard", [plan.shard_pad, hid], F8)
    h1_fa = nc.dram_tensor("h1_fa", [CORES * spl_r, hid], F8, addr_space="Shared")
    h1_fb = nc.dram_tensor("h1_fb", [CORES * plan.b_rows, hid], F8, addr_space="Shared")
    h2_sh = nc.dram_tensor("h2_shard", [plan.shard_pad, outc], F16)
    h2_fa = nc.dram_tensor("h2_fa", [CORES * spl_r, outc], F16, addr_space="Shared")
    h2_fb = nc.dram_tensor("h2_fb", [CORES * plan.b_rows, outc], F16, addr_space="Shared")

    st = contextlib.ExitStack()
    idx_sb = st.enter_context(nc.sbuf_tensor("idx_sb", [P, plan.tot_idxcols], I16))
    dr_sb = st.enter_context(nc.sbuf_tensor("dr_sb", [P, plan.tot_planes], F32))
    io_sb = st.enter_context(nc.sbuf_tensor("io_sb", [P, P], F32))
    id_sb = st.enter_context(nc.sbuf_tensor("id_sb", [P, P], F16))
    w1_sb = st.enter_context(nc.sbuf_tensor("w1_sb", [P, ncc, hid], F16))
    w2_sb = st.enter_context(nc.sbuf_tensor("w2_sb", [P, hcc, outc], F16))
    b1_sb = st.enter_context(nc.sbuf_tensor("b1_sb", [1, hid], F16))
    b2_sb = st.enter_context(nc.sbuf_tensor("b2_sb", [1, outc], F16))
    dv_sb = st.enter_context(nc.sbuf_tensor("dv_sb", [P, tiles], F32))
    sq_sb = st.enter_context(nc.sbuf_tensor("sq_sb", [1, plan.shard_pad], F16))

    nc.gpsimd.load_library(_mlp_lib)

    # one gpsimd register per distinct gather count
    _regs = {}

    def _nreg(v):
        if v not in _regs:
            _regs[v] = nc.gpsimd.to_reg(v)
        return _regs[v]

    for g in plan.groups:
        for sec in (0, 1):
            n = g["secs"][sec]["nch"] * P
            for off in range(0, n, GCAP):
                _nreg(min(GCAP, n - off))

    _qctr = [0]

    def _emit_gather(msg, ch0, nch, table, ic0, elem):
        """Gather nch*P rows into msg[:, ch0:ch0+nch, :] in <=GCAP pieces.

        Calls round-robin over the SWDGE queues so descriptor generation
        runs on all four Q7 DSP pairs concurrently."""
        n = nch * P
        ic = ic0
        for off in range(0, n, GCAP):
            nn = min(GCAP, n - off)
            nc.gpsimd.dma_gather(
                msg[:, ch0 + off // P : ch0 + (off + nn) // P, :],
                table,
                idx_sb[:, ic : ic + nn // 16],
                nn,
                _nreg(nn),
                elem,
                queue_num=_qctr[0] % NQ,
            )
            _qctr[0] += 1
            ic += nn // 16
        return ic

    def _emit_group_gathers(g, msg, ta, tb, elem):
        ic = g["idxcol_base"]
        an = g["secs"][0]["nch"]
        bn = g["secs"][1]["nch"]
        if an:
            ic = _emit_gather(msg, 0, an, ta, ic, elem)
        if bn:
            _emit_gather(msg, an, bn, tb, ic, elem)

    def _emit_agg(g, msg, loc, sB, psPool, width, b_sb, ptag):
        """Per-tile one-hot aggregation matmuls for group g (+ self-loop
        identity matmul over the local h' rows + rank-1 bias).

        Returns list of (tile_global, psum_tile)."""
        out = []
        for k, t in enumerate(g["tiles"]):
            planes = []  # (plane_col absolute, chunk col within group msg)
            for sec in (0, 1):
                S = g["secs"][sec]
                for pl, j in S["tile_planes"][k]:
                    planes.append(
                        (
                            g["plane_base"] + S["plane_off"] + pl,
                            S["chunk_off"] + j,
                        )
                    )
            npl = len(planes)
            Stile = sB.tile([P, max(npl, 1), P], F16, tag="S")
            pos = 0
            for sec in (0, 1):
                S = g["secs"][sec]
                tp = S["tile_planes"][k]
                if not tp:
                    continue
                c0 = g["plane_base"] + S["plane_off"] + tp[0][0]
                n = len(tp)
                assert tp[-1][0] - tp[0][0] == n - 1  # contiguous
                nc.vector.tensor_tensor(
                    out=Stile[:, pos : pos + n, :],
                    in0=io_sb[:, None, :].to_broadcast([P, n, P]),
                    in1=dr_sb[:, c0 : c0 + n][:, :, None].to_broadcast(
                        [P, n, P]
                    ),
                    op=mybir.AluOpType.is_equal,
                )
                pos += n
            ps = psPool.tile([P, width], F32, tag=ptag)
            for i, (_pl, ch) in enumerate(planes):
                nc.tensor.matmul(
                    ps[:, :],
                    lhsT=Stile[:, i, :],
                    rhs=msg[:, ch, :],
                    start=(i == 0),
                    stop=False,
                )
            # self-loop: += dinv[d]*h[d] (local h' rows, identity matmul)
            nc.tensor.matmul(
                ps[:, :],
                lhsT=id_sb[:, :],
                rhs=loc[:, k, :],
                start=(npl == 0),
                stop=False,
            )
            # rank-1 bias: += sqrt(deg)[n] * b[f]
            nc.tensor.matmul(
                ps[:, :],
                lhsT=sq_sb[0:1, t * P : (t + 1) * P],
                rhs=b_sb[0:1, :],
                start=False,
                stop=True,
            )
            out.append((t, ps))
        return out

    # ctx0: resident loads (own context so its exit barrier orders them
    # before every later consumer on every engine).
    with tile.TileContext(nc):
        nc.sync.dma_start(idx_sb[:, :], ix_d[:, :])
        nc.sync.dma_start(dr_sb[:, :], dr_d[:, :])
        nc.sync.dma_start(io_sb[:, :], io_d[:, :])
        nc.sync.dma_start(id_sb[:, :], id_d[:, :])
        nc.sync.dma_start(w1_sb[:, :, :], w1_d[:, :, :])
        nc.sync.dma_start(w2_sb[:, :, :], w2_d[:, :, :])
        nc.sync.dma_start(b1_sb[:, :], b1_d[:, :])
        nc.sync.dma_start(b2_sb[:, :], b2_d[:, :])
        nc.sync.dma_start(dv_sb[:, :], dv_d[:, :])
        nc.sync.dma_start(sq_sb[:, :], sq_d[:, :])

    # Fused context: dense projection, chunked AllGathers, both
    # aggregation layers all in one TileContext so the scheduler overlaps
    # collectives and gathers with compute across phase boundaries.
    with tile.TileContext(nc) as tc:
      with (
        tc.tile_pool(name="pA", bufs=2) as pA,
        tc.tile_pool(name="msgB", bufs=3) as msgB,
        tc.tile_pool(name="locB", bufs=2) as locB,
        tc.tile_pool(name="sB", bufs=2) as sB,
        tc.tile_pool(name="zB", bufs=2) as zB,
        tc.tile_pool(name="h2B", bufs=3) as h2B,
        tc.tile_pool(name="psB", bufs=3, space="PSUM") as psB,
        tc.tile_pool(name="psT", bufs=2, space="PSUM") as psT,
        tc.tile_pool(name="psH", bufs=3, space="PSUM") as psH,
        tc.tile_pool(name="msgC", bufs=4) as msgC,
        tc.tile_pool(name="locC", bufs=2) as locC,
        tc.tile_pool(name="sC", bufs=3) as sC,
        tc.tile_pool(name="oC", bufs=3) as oC,
      ):
        psA = psB   # same [P, hid] f32 tiles; stay within 8 PSUM banks
        psC = psH   # same [P, outc] f32 tiles
        # layer-1 dense projection h1' = D^-1/2 (x @ W1), 2 tiles/step;
        # each half's h1 rows AllGather as soon as they are written.
        for hi, (s0, s1) in enumerate([(0, spl_t), (spl_t, tiles)]):
                for t0 in range(s0, s1, 4):
                    nt = min(4, s1 - t0)
                    xa = pA.tile([P, nt, ncc, P], F16, tag="xa")
                    nc.sync.dma_start(xa[:, :, :, :], xt_d[t0 : t0 + nt, :, :, :].rearrange("t p c n -> p t c n"))
                    h1t = pA.tile([P, nt, hid], F8, tag="h1t")
                    for k in range(nt):
                        ps = psA.tile([P, hid], F32, tag="psAgg")
                        for cc in range(ncc):
                            nc.tensor.matmul(
                                ps[:, :],
                                lhsT=xa[:, k, cc, :],
                                rhs=w1_sb[:, cc, :],
                                start=(cc == 0),
                                stop=(cc == ncc - 1),
                            )
                        nc.scalar.activation(
                            h1t[:, k, :],
                            ps[:, :],
                            mybir.ActivationFunctionType.Copy,
                            scale=dv_sb[:, t0 + k : t0 + k + 1],
                        )
                    nc.sync.dma_start(
                        h1_sh[t0 * P : (t0 + nt) * P, :].rearrange(
                            "(t p) f -> p t f", p=P
                        ),
                        h1t[:, :, :],
                    )
                nc.gpsimd.collective_compute(
                    "AllGather",
                    mybir.AluOpType.bypass,
                    replica_groups=[list(range(CORES))],
                    ins=[h1_sh[s0 * P : s1 * P, :].bitcast(F32)],
                    outs=[(h1_fa if hi == 0 else h1_fb)[:, :].bitcast(F32)],
                )

        # layer-1 aggregate + relu, then layer-2 dense projection; the
        # first part's h2 rows AllGather while the second part computes.
        gsplit = spl_t // GRP
        assert plan.groups[gsplit]["tiles"][0] == spl_t
        for hi, (gs0, gs1) in enumerate([(0, gsplit), (gsplit, len(plan.groups))]):
            for g in plan.groups[gs0:gs1]:
                nt = len(g["tiles"])
                t0 = g["tiles"][0]
                msg = msgB.tile([P, g["nch"], hid], F8, tag="msg")
                _emit_group_gathers(g, msg, h1_fa[:, :], h1_fb[:, :], hid)
                loc = locB.tile([P, nt, hid], F8, tag="loc")
                nc.sync.dma_start(
                    loc[:, :, :],
                    h1_sh[t0 * P : (t0 + nt) * P, :].rearrange(
                        "(t p) f -> p t f", p=P
                    ),
                )
                for t, ps in _emit_agg(g, msg, loc, sB, psB, hid, b1_sb, "psAgg"):
                    # z1 = relu(dinv[n] * ps)  [node, f] fp16
                    z1 = zB.tile([P, hid], F16, tag="z1")
                    nc.vector.tensor_scalar(
                        out=z1[:, :],
                        in0=ps[:, :],
                        scalar1=dv_sb[:, t : t + 1],
                        scalar2=0.0,
                        op0=mybir.AluOpType.mult,
                        op1=mybir.AluOpType.max,
                    )
                    # transpose z1 -> z1T for the layer-2 contraction
                    zt_ps = psT.tile([P, hcc, P], F16, tag="ztps")
                    for h in range(hcc):
                        nc.tensor.transpose(
                            zt_ps[:, h, :],
                            z1[:, h * P : (h + 1) * P],
                            id_sb[:, :],
                        )
                    zt = zB.tile([P, hcc, P], F16, tag="zt")
                    nc.scalar.activation(
                        zt[:, :, :],
                        zt_ps[:, :, :],
                        mybir.ActivationFunctionType.Copy,
                    )
                    hps = psH.tile([P, outc], F32, tag="hps")
                    for cc in range(hcc):
                        nc.tensor.matmul(
                            hps[:, :],
                            lhsT=zt[:, cc, :],
                            rhs=w2_sb[:, cc, :],
                            start=(cc == 0),
                            stop=(cc == hcc - 1),
                        )
                    # h2' = dinv[n] * (z1 @ W2)
                    h2t = h2B.tile([P, outc], F16, tag="h2t")
                    nc.scalar.activation(
                        h2t[:, :],
                        hps[:, :],
                        mybir.ActivationFunctionType.Copy,
                        scale=dv_sb[:, t : t + 1],
                    )
                    nc.sync.dma_start(h2_sh[t * P : (t + 1) * P, :], h2t[:, :])
            r0 = plan.groups[gs0]["tiles"][0] * P
            r1 = (plan.groups[gs1 - 1]["tiles"][-1] + 1) * P
            nc.gpsimd.collective_compute(
                "AllGather",
                mybir.AluOpType.bypass,
                replica_groups=[list(range(CORES))],
                ins=[h2_sh[r0:r1, :].bitcast(F32)],
                outs=[(h2_fa if hi == 0 else h2_fb)[:, :].bitcast(F32)],
            )

        # layer-2 aggregate + bias -> output
        if True:
            for g in plan.groups:
                nt = len(g["tiles"])
                t0 = g["tiles"][0]
                msg = msgC.tile([P, g["nch"], outc], F16, tag="msg2")
                _emit_group_gathers(g, msg, h2_fa[:, :], h2_fb[:, :], outc)
                loc = locC.tile([P, nt, outc], F16, tag="loc2")
                nc.sync.dma_start(
                    loc[:, :, :],
                    h2_sh[t0 * P : (t0 + nt) * P, :].rearrange(
                        "(t p) f -> p t f", p=P
                    ),
                )
                for t, ps in _emit_agg(g, msg, loc, sC, psC, outc, b2_sb, "hps"):
                    ob = oC.tile([P, outc], F32, tag="ob")
                    nc.vector.tensor_scalar(
                        out=ob[:, :],
                        in0=ps[:, :],
                        scalar1=dv_sb[:, t : t + 1],
                        scalar2=None,
                        op0=mybir.AluOpType.mult,
                    )
                    nc.sync.dma_start(out_d[t * P : (t + 1) * P, :], ob[:, :])

    st.close()
    nc.compile()
    return nc


def _make_in_maps(plan, x, W1, b1, W2, b2):
    inc, hid, outc = plan.inc, plan.hid, plan.outc
    ncc, hcc = inc // P, hid // P
    w1t = np.ascontiguousarray(
        W1.reshape(ncc, P, hid).transpose(1, 0, 2).astype(np.float16)
    )
    w2t = np.ascontiguousarray(
        W2.reshape(hcc, P, outc).transpose(1, 0, 2).astype(np.float16)
    )
    b1r = np.ascontiguousarray(b1.astype(np.float16).reshape(1, hid))
    b2r = np.ascontiguousarray(b2.astype(np.float16).reshape(1, outc))
    iota = np.ascontiguousarray(
        np.tile(np.arange(P, dtype=np.float32), (P, 1))
    )
    ident = np.ascontiguousarray(np.eye(P, dtype=np.float16))
    in_maps = []
    for c in range(CORES):
        xs = x[c * plan.shard : (c + 1) * plan.shard].astype(np.float32)
        xs = np.pad(xs, ((0, plan.shard_pad - plan.shard), (0, 0)))
        xt = xs.reshape(plan.tiles, P, ncc, P).transpose(0, 3, 2, 1)
        in_maps.append(
            {
                "xt": np.ascontiguousarray(xt.astype(np.float16)),
                "w1t": w1t,
                "w2t": w2t,
                "b1r": b1r,
                "b2r": b2r,
                "iota": iota,
                "ident": ident,
                "idx": plan.core_idx[c],
                "dstrel": plan.core_drel[c],
                "dinvc": plan.core_dinvc[c],
                "sqd": plan.core_sqd[c],
            }
        )
    return in_maps


_CACHE = {}


def _get_built(x, edge_index, W1, b1, W2, b2):
    n_nodes, in_c = x.shape
    hid = W1.shape[1]
    out_c = W2.shape[1]
    key = (n_nodes, in_c, hid, out_c, hash(edge_index.tobytes()))
    if key not in _CACHE:
        src = np.asarray(edge_index[0], np.int64)
        dst = np.asarray(edge_index[1], np.int64)
        plan = Plan(n_nodes, src, dst, in_c, hid, out_c)
        nc = _build_nc(plan)
        _CACHE[key] = (plan, nc)
    return _CACHE[key]


def run(x, edge_index, W1, b1, W2, b2, trace=False, **spmd_kwargs):
    plan, nc = _get_built(x, edge_index, W1, b1, W2, b2)
    in_maps = _make_in_maps(plan, x, W1, b1, W2, b2)
    res = run_bass_kernel_spmd(
        nc, in_maps, core_ids=list(range(CORES)), trace=trace, **spmd_kwargs
    )
    out = np.concatenate(
        [res.results[c]["out"][: plan.shard] for c in range(CORES)], axis=0
    ).astype(np.float32)
    return out, res


def kernel(**inputs):
    x = np.asarray(inputs["x"], np.float32)
    edge_index = np.asarray(inputs["edge_index"])
    W1 = np.asarray(inputs["W1"], np.float32)
    b1 = np.asarray(inputs["b1"], np.float32)
    W2 = np.asarray(inputs["W2"], np.float32)
    b2 = np.asarray(inputs["b2"], np.float32)
    out, _ = run(x, edge_index, W1, b1, W2, b2)
    return out

